# revision 61
# baseline (speedup 1.0000x reference)
"""GRU-D cell on 8 Trainium2 NeuronCores (Bass/Tile SPMD kernel).

Data-parallel: the batch dim (16384) is sharded 8 x 2048 across cores;
the six 512x512 weight matrices are replicated. Per core, the 2048-row
shard is processed as 16 tiles of 128 rows (batch on partitions):

  gamma   = exp(-relu(gamma_decay) * delta_t)          (ACT, fused scale)
  x_dec   = x * (m + gx - m*gx)  [+ mu * (1 - q)]      (DVE, bf16)
  h_dec   = gh * h_prev                                (DVE)
  z/r/hh  : PE matmuls, contraction dim on partitions via PE transposes
            of x_dec / h_dec / (r*h_dec); weights replicated in SBUF bf16
  h_new   = h_dec + z * (tanh(hh_pre) - h_dec)         (DVE, fp32 out)

Inputs are cast to bf16 on the host (round-to-nearest) to halve HBM
traffic and let the PE run at full bf16 rate; accumulation stays fp32 in
PSUM. Zero biases / zero mean-imputation / all-ones decay vectors (the
values this problem ships) are detected at runtime and the matching
pre-compiled specialization is used; non-trivial values fall back to a
general variant built from the same emitter.
"""

import os
import threading
from contextlib import ExitStack

import ml_dtypes
import numpy as np

F = 512
P = 128
N_CORES = 8
B = 16384
BC = B // N_CORES          # rows per core
NT = BC // P               # 128-row tiles per core
KC = F // P                # contraction chunks

BF16 = ml_dtypes.bfloat16

_lock = threading.Lock()
_cache = {}

# Tunables (settled via timeline-sim scans + HW loop-delta timing).
_TUNE = {
    "hd_eng": "vector",
    "rh_eng": "vector",
    "xdT_eng": "vector",
    "hdT_eng": "scalar",
    "rhT_eng": "vector",
    "wk_bufs": 5,
    "io_bufs": 3,
    "pt_bufs": 4,
    "pg_bufs": 4,
    "offs": (0, 1, 2, 3, 4),
    "zaff_eng": "gpsimd",
    "d_eng": "vector",
    "zd_eng": "vector",
    "w_split": True,
    "warmup": 24,
    # layout B knobs
    "layout": "A",
    "wkb_bufs": 3,
    "iob_bufs": 3,
    "pgb_bufs": 6,
    "ptb_bufs": 2,
    "offs_b": (0, 1, 2, 3),
    "hnc_eng": "scalar",
}


def _HD_ENG(nc):
    return getattr(nc, _TUNE["hd_eng"])


def _RH_ENG(nc):
    return getattr(nc, _TUNE["rh_eng"])


def _copy(nc, eng, out, in_):
    if eng == "scalar":
        nc.scalar.copy(out, in_)
    else:
        getattr(nc, eng).tensor_copy(out, in_)


def _to_bf16(a: np.ndarray) -> np.ndarray:
    """Fast fp32 -> bf16 with round-to-nearest (ties toward +1 ulp)."""
    a = np.ascontiguousarray(a, dtype=np.float32)
    u = a.view(np.uint32)
    r = (u >> 16) & np.uint32(1)
    out = ((u + np.uint32(0x7FFF) + r) >> 16).astype(np.uint16)
    return out.view(BF16).reshape(a.shape)


def _build_nc(flags, reps=1):
    if _TUNE.get("layout", "A") == "B":
        return _build_nc_b(flags, reps)
    return _build_nc_a(flags, reps)


def _build_nc_a(flags, reps=1):
    """Emit + compile the Bass program for one specialization.

    flags = (shared_gamma, gx_one, gh_one, mu_zero, bz_zero, br_zero, bh_zero)
    reps > 1 unrolls the whole pass multiple times in one NEFF — used only
    for device-time measurement (loop-delta timing amortizes dispatch).
    Returns (nc, in_names_needed).
    """
    import concourse.bass as bass
    import concourse.tile as tile
    from concourse import bacc, mybir
    from concourse.bass import ts
    from concourse.masks import make_identity

    shared_gamma, gx_one, gh_one, mu_zero, bz_zero, br_zero, bh_zero = flags
    BD = mybir.dt.bfloat16
    FD = mybir.dt.float32
    AF = mybir.ActivationFunctionType

    nc = bacc.Bacc("TRN2", target_bir_lowering=False, debug=False,
                   num_devices=N_CORES)

    inp = nc.dram_tensor("inp", [BC, 3 * F], BD, kind="ExternalInput").ap()
    hp = nc.dram_tensor("hp", [BC, F], BD, kind="ExternalInput").ap()
    w = nc.dram_tensor("w", [6, F, F], BD, kind="ExternalInput").ap()
    need = ["inp", "hp", "w"]
    gxr = ghr = mu = bz = br = bh = None
    if not gx_one:
        gxr = nc.dram_tensor("gxr", [F], BD, kind="ExternalInput").ap()
        need.append("gxr")
    if not shared_gamma and not gh_one:
        ghr = nc.dram_tensor("ghr", [F], BD, kind="ExternalInput").ap()
        need.append("ghr")
    if not mu_zero:
        mu = nc.dram_tensor("mu", [F], BD, kind="ExternalInput").ap()
        need.append("mu")
    if not bz_zero:
        bz = nc.dram_tensor("bz", [F], FD, kind="ExternalInput").ap()
        need.append("bz")
    if not br_zero:
        br = nc.dram_tensor("br", [F], FD, kind="ExternalInput").ap()
        need.append("br")
    if not bh_zero:
        bh = nc.dram_tensor("bh", [F], FD, kind="ExternalInput").ap()
        need.append("bh")
    out = nc.dram_tensor("h_new", [BC, F], FD, kind="ExternalOutput").ap()

    inp_r = inp.rearrange("(n p) c -> n p c", p=P)
    hp_r = hp.rearrange("(n p) c -> n p c", p=P)
    out_r = out.rearrange("(n p) c -> n p c", p=P)
    w_r = w.rearrange("w (k p) u -> p w k u", p=P)

    def bcast(pool, dram_ap, dt, name):
        t = pool.tile([P, F], dt, name=name)
        src = bass.AP(tensor=dram_ap.tensor, offset=dram_ap.offset,
                      ap=[[0, P]] + [list(d) for d in dram_ap.ap])
        nc.gpsimd.dma_start(out=t, in_=src)
        return t

    with tile.TileContext(nc) as tc:
        with ExitStack() as ctx:
            consts = ctx.enter_context(tc.tile_pool(name="consts", bufs=1))
            wk = ctx.enter_context(
                tc.tile_pool(name="wk", bufs=_TUNE["wk_bufs"]))
            io_pool = ctx.enter_context(
                tc.tile_pool(name="io", bufs=_TUNE["io_bufs"]))
            pt = ctx.enter_context(
                tc.tile_pool(name="pt", bufs=_TUNE["pt_bufs"], space="PSUM"))
            pg = ctx.enter_context(
                tc.tile_pool(name="pg", bufs=_TUNE["pg_bufs"], space="PSUM"))

            ident = consts.tile([P, P], BD)
            make_identity(nc, ident)
            w_sb = consts.tile([P, 6, KC, F], BD)
            pending_w = {}
            if _TUNE["w_split"]:
                # Weight loads are emitted into the early pipeline rounds
                # (after the first input tiles) so the first transposes and
                # matmuls aren't gated on the full 3MB weight load.
                pending_w = {0: [0, 1], 1: [2, 3], 2: [4, 5]}
            else:
                nc.sync.dma_start(out=w_sb, in_=w_r)
            if _TUNE["warmup"]:
                # The PE clock ramps with sustained use (full speed only
                # after ~3us busy). Run throwaway transposes on the identity
                # during the initial DMA fill so the first real matmuls start
                # at full clock. (Reuses a pt-pool slot; WAW on one tile
                # serializes them on the PE, which is exactly the point.)
                wu_p = pt.tile([P, P], BD, tag="tp", name="wu_p")
                for _ in range(_TUNE["warmup"]):
                    nc.tensor.transpose(wu_p, ident, ident)
            gxb = ghb = mub = bzb = brb = bhb = None
            if gxr is not None:
                gxb = bcast(consts, gxr, BD, "gxb")
            if ghr is not None:
                ghb = bcast(consts, ghr, BD, "ghb")
            if mu is not None:
                mub = bcast(consts, mu, BD, "mub")
            if bz is not None:
                bzb = bcast(consts, bz, FD, "bzb")
            if br is not None:
                brb = bcast(consts, br, FD, "brb")
            if bh is not None:
                bhb = bcast(consts, bh, FD, "bhb")

            if reps > 1:
                loop_ctx = tc.For_i(0, reps, 1)
                loop_ctx.__enter__()

            def stage_a1(i):
                """DMA-in, imputation/decay, xd/hd transposes."""
                it = io_pool.tile([P, 3 * F], BD, tag="it")
                if i == 0:
                    # delta_t slice first: it gates the exp that heads the
                    # whole dependency chain of the first tile.
                    nc.sync.dma_start(out=it[:, 2 * F:], in_=inp_r[i][:, 2 * F:])
                    nc.sync.dma_start(out=it[:, :2 * F], in_=inp_r[i][:, :2 * F])
                else:
                    nc.sync.dma_start(out=it, in_=inp_r[i])
                ht = io_pool.tile([P, F], BD, tag="ht")
                nc.sync.dma_start(out=ht, in_=hp_r[i])
                x_ = it[:, 0:F]
                m_ = it[:, F:2 * F]
                dt_ = it[:, 2 * F:3 * F]

                # gamma_x = exp(-relu(gx) * delta_t)
                g = wk.tile([P, F], BD, tag="g")
                if gx_one:
                    nc.scalar.activation(g, dt_, AF.Exp, scale=-1.0)
                else:
                    ga = wk.tile([P, F], BD, tag="ga")
                    nc.vector.tensor_mul(ga, dt_, gxb)
                    nc.scalar.activation(g, ga, AF.Exp, scale=-1.0)
                if shared_gamma:
                    gH = g
                elif gh_one:
                    gH = wk.tile([P, F], BD, tag="gH")
                    nc.scalar.activation(gH, dt_, AF.Exp, scale=-1.0)
                else:
                    gha = wk.tile([P, F], BD, tag="gha")
                    nc.vector.tensor_mul(gha, dt_, ghb)
                    gH = wk.tile([P, F], BD, tag="gH")
                    nc.scalar.activation(gH, gha, AF.Exp, scale=-1.0)

                # x_decayed = x*q (+ mu*(1-q)), q = m + g - m*g
                t1 = wk.tile([P, F], BD, tag="t1")
                nc.vector.tensor_mul(t1, m_, g)
                q = wk.tile([P, F], BD, tag="q")
                nc.vector.tensor_add(q, m_, g)
                q2 = wk.tile([P, F], BD, tag="q2")
                nc.vector.tensor_sub(q2, q, t1)
                xd = wk.tile([P, F], BD, tag="xd")
                nc.vector.tensor_mul(xd, x_, q2)
                if not mu_zero:
                    omq = wk.tile([P, F], BD, tag="omq")
                    nc.vector.tensor_scalar(
                        omq, q2, -1.0, 1.0,
                        mybir.AluOpType.mult, mybir.AluOpType.add)
                    muq = wk.tile([P, F], BD, tag="muq")
                    nc.vector.tensor_mul(muq, omq, mub)
                    xd2 = wk.tile([P, F], BD, tag="xd2")
                    nc.vector.tensor_add(xd2, xd, muq)
                    xd = xd2
                hd = wk.tile([P, F], BD, tag="hd")
                _HD_ENG(nc).tensor_mul(hd, gH, ht)
                return dict(xd=xd, hd=hd, i=i)

            def stage_tr(st):
                """PE transposes of xd/hd: [128-batch, 512-f] -> [512-f, 128-b]."""
                xd, hd = st["xd"], st["hd"]
                xdT_p = pt.tile([P, F], BD, tag="tp")
                for k in range(KC):
                    nc.tensor.transpose(xdT_p[:, ts(k, P)], xd[:, ts(k, P)], ident)
                xdT = wk.tile([P, F], BD, tag="xdT")
                _copy(nc, _TUNE["xdT_eng"], xdT, xdT_p)
                hdT_p = pt.tile([P, F], BD, tag="tp")
                for k in range(KC):
                    nc.tensor.transpose(hdT_p[:, ts(k, P)], hd[:, ts(k, P)], ident)
                hdT = wk.tile([P, F], BD, tag="hdT")
                _copy(nc, _TUNE["hdT_eng"], hdT, hdT_p)
                st.update(xdT=xdT, hdT=hdT)
                return st

            def stage_zr(st):
                """z/r gate matmuls, sigmoids, rh."""
                xdT, hdT, hd = st["xdT"], st["hdT"], st["hd"]
                zp = pg.tile([P, F], FD, tag="gp")
                for k in range(KC):
                    nc.tensor.matmul(zp, xdT[:, ts(k, P)], w_sb[:, 0, k, :],
                                     start=(k == 0), stop=False)
                for k in range(KC):
                    nc.tensor.matmul(zp, hdT[:, ts(k, P)], w_sb[:, 1, k, :],
                                     start=False, stop=(k == KC - 1))
                rp = pg.tile([P, F], FD, tag="gp")
                for k in range(KC):
                    nc.tensor.matmul(rp, xdT[:, ts(k, P)], w_sb[:, 2, k, :],
                                     start=(k == 0), stop=False)
                for k in range(KC):
                    nc.tensor.matmul(rp, hdT[:, ts(k, P)], w_sb[:, 3, k, :],
                                     start=False, stop=(k == KC - 1))
                # sigmoid(a) = 0.5 + 0.5*tanh(a/2): keeps every ACT func in
                # the single `exp_and_others` table (Exp/Tanh/Copy) — a
                # Sigmoid would force ~1.3us act-table reloads per switch.
                zt = wk.tile([P, F], BD, tag="zt")
                if bz_zero:
                    nc.scalar.activation(zt, zp, AF.Tanh, scale=0.5)
                else:
                    zb = wk.tile([P, F], FD, tag="zb")
                    nc.vector.tensor_add(zb, zp, bzb)
                    nc.scalar.activation(zt, zb, AF.Tanh, scale=0.5)
                z = wk.tile([P, F], BD, tag="z")
                getattr(nc, _TUNE["zaff_eng"]).tensor_scalar(
                    z, zt, 0.5, 0.5,
                    mybir.AluOpType.mult, mybir.AluOpType.add)
                rt = wk.tile([P, F], BD, tag="rt")
                if br_zero:
                    nc.scalar.activation(rt, rp, AF.Tanh, scale=0.5)
                else:
                    rb = wk.tile([P, F], FD, tag="rb")
                    nc.vector.tensor_add(rb, rp, brb)
                    nc.scalar.activation(rt, rb, AF.Tanh, scale=0.5)
                # r = 0.5 + 0.5*rt, and rh = r*hd feeds only (rh @ U_h).
                # The 0.5 factor is folded into U_h host-side, so
                # rh' = hd + rt*hd  (saves the affine op on r).
                v = wk.tile([P, F], BD, tag="v")
                nc.vector.tensor_mul(v, rt, hd)
                rh = wk.tile([P, F], BD, tag="rh")
                _RH_ENG(nc).tensor_add(rh, hd, v)
                st.update(rh=rh, z=z)
                return st

            def stage_rt(st):
                """rh transpose."""
                rh = st["rh"]
                rhT_p = pt.tile([P, F], BD, tag="tp")
                for k in range(KC):
                    nc.tensor.transpose(rhT_p[:, ts(k, P)], rh[:, ts(k, P)], ident)
                rhT = wk.tile([P, F], BD, tag="rhT")
                _copy(nc, _TUNE["rhT_eng"], rhT, rhT_p)
                st.update(rhT=rhT)
                return st

            def stage_b(st):
                """hh gate matmuls, tanh, blend, DMA-out."""
                xdT, rhT, hd, z, i = (st["xdT"], st["rhT"], st["hd"],
                                      st["z"], st["i"])
                hpp = pg.tile([P, F], FD, tag="gp")
                for k in range(KC):
                    nc.tensor.matmul(hpp, xdT[:, ts(k, P)], w_sb[:, 4, k, :],
                                     start=(k == 0), stop=False)
                for k in range(KC):
                    nc.tensor.matmul(hpp, rhT[:, ts(k, P)], w_sb[:, 5, k, :],
                                     start=False, stop=(k == KC - 1))
                hh = wk.tile([P, F], BD, tag="hh")
                if bh_zero:
                    nc.scalar.activation(hh, hpp, AF.Tanh)
                else:
                    hb = wk.tile([P, F], FD, tag="hb")
                    nc.vector.tensor_add(hb, hpp, bhb)
                    nc.scalar.activation(hh, hb, AF.Tanh)

                # h_new = hd + z*(hh - hd)
                d = wk.tile([P, F], BD, tag="d")
                getattr(nc, _TUNE["d_eng"]).tensor_sub(d, hh, hd)
                zd = wk.tile([P, F], BD, tag="zd")
                getattr(nc, _TUNE["zd_eng"]).tensor_mul(zd, z, d)
                hn = wk.tile([P, F], FD, tag="hn")
                nc.vector.tensor_add(hn, hd, zd)
                nc.sync.dma_start(out=out_r[i], in_=hn)

            stages = [stage_a1, stage_tr, stage_zr, stage_rt, stage_b]
            offs = _TUNE["offs"]
            sts = {}
            for r in range(NT + max(offs)):
                for s, off in zip(stages, offs):
                    j = r - off
                    if 0 <= j < NT:
                        if s is stage_a1:
                            sts[j] = s(j)
                        else:
                            s(sts[j])
                for j in pending_w.pop(r, ()):
                    nc.sync.dma_start(out=w_sb[:, j], in_=w_r[:, j])
            sts.clear()

            if reps > 1:
                loop_ctx.__exit__(None, None, None)

    nc.compile()
    return nc, need


def _build_nc_b(flags, reps=1):
    """Layout B: feature-major compute via DMA-transposed input loads.

    Inputs land in SBUF already transposed ([feature-part, batch-free],
    512-row batch chunks), so the imputation/decay math, gates and blend
    all run feature-major: no PE input transposes, no PSUM->SBUF copies
    for matmul operands, and biases/decay vectors fuse into ACT as
    per-partition scalars. Only the final h_new needs PE transposes back
    to batch-major. Weights are the stationary matmul operand here.
    """
    import concourse.bass as bass
    import concourse.tile as tile
    from concourse import bacc, mybir
    from concourse.bass import ts
    from concourse.masks import make_identity

    shared_gamma, gx_one, gh_one, mu_zero, bz_zero, br_zero, bh_zero = flags
    BD = mybir.dt.bfloat16
    FD = mybir.dt.float32
    AF = mybir.ActivationFunctionType
    CW = 512               # batch columns per chunk
    NCH = BC // CW         # chunks per core (4)

    nc = bacc.Bacc("TRN2", target_bir_lowering=False, debug=False,
                   num_devices=N_CORES)

    inp = nc.dram_tensor("inp", [BC, 3 * F], BD, kind="ExternalInput").ap()
    hp = nc.dram_tensor("hp", [BC, F], BD, kind="ExternalInput").ap()
    w = nc.dram_tensor("w", [6, F, F], BD, kind="ExternalInput").ap()
    need = ["inp", "hp", "w"]
    gxr = ghr = mu = bz = br = bh = None
    if not gx_one:
        # host passes NEGATED relu'd decay (used as ACT scale)
        gxr = nc.dram_tensor("gxr", [F], FD, kind="ExternalInput").ap()
        need.append("gxr")
    if not shared_gamma and not gh_one:
        ghr = nc.dram_tensor("ghr", [F], FD, kind="ExternalInput").ap()
        need.append("ghr")
    if not mu_zero:
        mu = nc.dram_tensor("mu", [F], FD, kind="ExternalInput").ap()
        need.append("mu")
    if not bz_zero:
        bz = nc.dram_tensor("bz", [F], FD, kind="ExternalInput").ap()
        need.append("bz")
    if not br_zero:
        br = nc.dram_tensor("br", [F], FD, kind="ExternalInput").ap()
        need.append("br")
    if not bh_zero:
        bh = nc.dram_tensor("bh", [F], FD, kind="ExternalInput").ap()
        need.append("bh")
    out = nc.dram_tensor("h_new", [BC, F], FD, kind="ExternalOutput").ap()

    w_r = w.rearrange("w (k p) u -> p w k u", p=P)

    def fmaj(pool, dram_ap, name):
        # [F] vector -> [128, KC] feature-major ([p, c] = v[c*128+p])
        t = pool.tile([P, KC], FD, name=name)
        nc.gpsimd.dma_start(out=t, in_=dram_ap.rearrange("(c p) -> p c", p=P))
        return t

    with tile.TileContext(nc) as tc:
        with ExitStack() as ctx:
            consts = ctx.enter_context(tc.tile_pool(name="consts", bufs=1))
            wk = ctx.enter_context(
                tc.tile_pool(name="wk", bufs=_TUNE["wkb_bufs"]))
            io_pool = ctx.enter_context(
                tc.tile_pool(name="io", bufs=_TUNE["iob_bufs"]))
            pg = ctx.enter_context(
                tc.tile_pool(name="pg", bufs=_TUNE["pgb_bufs"], space="PSUM"))
            pt = ctx.enter_context(
                tc.tile_pool(name="pt", bufs=_TUNE["ptb_bufs"], space="PSUM"))

            ident = consts.tile([P, P], BD)
            make_identity(nc, ident)
            w_sb = consts.tile([P, 6, KC, F], BD)
            pending_w = {0: [0, 1, 2, 3], 1: [4, 5]}

            gxb = ghb = mub = nmub = bzb = brb = bhb = None
            if gxr is not None:
                gxb = fmaj(consts, gxr, "gxb")
            if ghr is not None:
                ghb = fmaj(consts, ghr, "ghb")
            if mu is not None:
                mub = fmaj(consts, mu, "mub")
                nmub = consts.tile([P, KC], FD, name="nmub")
                nc.vector.tensor_scalar(nmub, mub, -1.0, None,
                                        mybir.AluOpType.mult)
            if bz is not None:
                bzb = fmaj(consts, bz, "bzb")   # host pre-scaled by 0.5
            if br is not None:
                brb = fmaj(consts, br, "brb")   # host pre-scaled by 0.5
            if bh is not None:
                bhb = fmaj(consts, bh, "bhb")

            def s_load(c):
                """DMA-transpose x/m/dt/h for batch rows [c*CW, (c+1)*CW)."""
                rows = slice(c * CW, (c + 1) * CW)
                xT = io_pool.tile([P, KC, CW], BD, tag="xT")
                mT = io_pool.tile([P, KC, CW], BD, tag="mT")
                dT = io_pool.tile([P, KC, CW], BD, tag="dT")
                hT = io_pool.tile([P, KC, CW], BD, tag="hT")
                # delta_t first (the exp is the first consumer), then m, x, h
                for f in range(KC):
                    nc.sync.dma_start(
                        out=dT[:, f, :],
                        in_=inp[rows, 2 * F + f * P:2 * F + (f + 1) * P],
                        transpose=True)
                for f in range(KC):
                    nc.sync.dma_start(out=mT[:, f, :],
                                      in_=inp[rows, F + f * P:F + (f + 1) * P],
                                      transpose=True)
                for f in range(KC):
                    nc.sync.dma_start(out=xT[:, f, :],
                                      in_=inp[rows, ts(f, P)], transpose=True)
                for f in range(KC):
                    nc.sync.dma_start(out=hT[:, f, :],
                                      in_=hp[rows, ts(f, P)], transpose=True)
                return dict(xT=xT, mT=mT, dT=dT, hT=hT, c=c)

            def s_imp(st):
                """Imputation + decay, feature-major, mostly in place:
                xT -> x_decayed, hT -> h_decayed, mT/dT scratch."""
                xT, mT, dT, hT = st["xT"], st["mT"], st["dT"], st["hT"]
                g = wk.tile([P, KC, CW], BD, tag="g")
                if gx_one:
                    nc.scalar.activation(g, dT, AF.Exp, scale=-1.0)
                else:
                    for f in range(KC):
                        nc.scalar.activation(g[:, f, :], dT[:, f, :], AF.Exp,
                                             scale=gxb[:, f:f + 1])
                if shared_gamma:
                    gH = g
                elif gh_one:
                    gH = wk.tile([P, KC, CW], BD, tag="gH")
                    nc.scalar.activation(gH, dT, AF.Exp, scale=-1.0)
                else:
                    gH = wk.tile([P, KC, CW], BD, tag="gH")
                    for f in range(KC):
                        nc.scalar.activation(gH[:, f, :], dT[:, f, :], AF.Exp,
                                             scale=ghb[:, f:f + 1])
                # q = m + g - m*g  (dT <- m*g, mT <- q)
                nc.vector.tensor_mul(dT, mT, g)
                nc.vector.tensor_add(mT, mT, g)
                nc.vector.tensor_sub(mT, mT, dT)
                # xd = x*q (+ mu*(1-q))
                nc.vector.tensor_mul(xT, xT, mT)
                if not mu_zero:
                    for f in range(KC):
                        nc.vector.tensor_scalar(
                            dT[:, f, :], mT[:, f, :],
                            nmub[:, f:f + 1], mub[:, f:f + 1],
                            mybir.AluOpType.mult, mybir.AluOpType.add)
                    nc.vector.tensor_add(xT, xT, dT)
                # hd = gH * h
                nc.vector.tensor_mul(hT, gH, hT)
                st["g"] = g
                return st

            def s_zr(st):
                """z and r gates + rh (in place on rt)."""
                xT, hT = st["xT"], st["hT"]
                zt = wk.tile([P, KC, CW], BD, tag="zt")
                for u in range(KC):
                    zp = pg.tile([P, CW], FD, tag="gp")
                    for f in range(KC):
                        nc.tensor.matmul(zp, w_sb[:, 0, f, ts(u, P)],
                                         xT[:, f, :],
                                         start=(f == 0), stop=False)
                    for f in range(KC):
                        nc.tensor.matmul(zp, w_sb[:, 1, f, ts(u, P)],
                                         hT[:, f, :],
                                         start=False, stop=(f == KC - 1))
                    if bz_zero:
                        nc.scalar.activation(zt[:, u, :], zp, AF.Tanh,
                                             scale=0.5)
                    else:
                        nc.scalar.activation(zt[:, u, :], zp, AF.Tanh,
                                             scale=0.5, bias=bzb[:, u:u + 1])
                z = wk.tile([P, KC, CW], BD, tag="z")
                getattr(nc, _TUNE["zaff_eng"]).tensor_scalar(
                    z, zt, 0.5, 0.5,
                    mybir.AluOpType.mult, mybir.AluOpType.add)
                rt = wk.tile([P, KC, CW], BD, tag="rt")
                for u in range(KC):
                    rp = pg.tile([P, CW], FD, tag="gp")
                    for f in range(KC):
                        nc.tensor.matmul(rp, w_sb[:, 2, f, ts(u, P)],
                                         xT[:, f, :],
                                         start=(f == 0), stop=False)
                    for f in range(KC):
                        nc.tensor.matmul(rp, w_sb[:, 3, f, ts(u, P)],
                                         hT[:, f, :],
                                         start=False, stop=(f == KC - 1))
                    if br_zero:
                        nc.scalar.activation(rt[:, u, :], rp, AF.Tanh,
                                             scale=0.5)
                    else:
                        nc.scalar.activation(rt[:, u, :], rp, AF.Tanh,
                                             scale=0.5, bias=brb[:, u:u + 1])
                # rh' = hd + rt*hd  (U_h host-scaled by 0.5); in place on rt
                nc.vector.tensor_mul(rt, rt, hT)
                nc.vector.tensor_add(rt, rt, hT)
                st.update(z=z, rh=rt)
                return st

            def s_hh(st):
                """hh gate, blend (in place -> h_new^T), transpose, store."""
                xT, hT, rh, z, c = st["xT"], st["hT"], st["rh"], st["z"], st["c"]
                hhT = wk.tile([P, KC, CW], BD, tag="hhT")
                for u in range(KC):
                    hpp = pg.tile([P, CW], FD, tag="gp")
                    for f in range(KC):
                        nc.tensor.matmul(hpp, w_sb[:, 4, f, ts(u, P)],
                                         xT[:, f, :],
                                         start=(f == 0), stop=False)
                    for f in range(KC):
                        nc.tensor.matmul(hpp, w_sb[:, 5, f, ts(u, P)],
                                         rh[:, f, :],
                                         start=False, stop=(f == KC - 1))
                    if bh_zero:
                        nc.scalar.activation(hhT[:, u, :], hpp, AF.Tanh)
                    else:
                        nc.scalar.activation(hhT[:, u, :], hpp, AF.Tanh,
                                             bias=bhb[:, u:u + 1])
                # h_new = hd + z*(hh - hd), in place on hhT
                nc.vector.tensor_sub(hhT, hhT, hT)
                nc.vector.tensor_mul(hhT, z, hhT)
                nc.vector.tensor_add(hhT, hT, hhT)
                # transpose back to batch-major and store
                for bs in range(CW // P):
                    hn_p = pt.tile([P, F], BD, tag="tp")
                    for u in range(KC):
                        nc.tensor.transpose(hn_p[:, ts(u, P)],
                                            hhT[:, u, ts(bs, P)], ident)
                    hn = wk.tile([P, F], FD, tag="hn")
                    _copy(nc, _TUNE["hnc_eng"], hn, hn_p)
                    nc.sync.dma_start(
                        out=out[c * CW + bs * P:c * CW + (bs + 1) * P, :],
                        in_=hn)

            if reps > 1:
                loop_ctx = tc.For_i(0, reps, 1)
                loop_ctx.__enter__()

            stages = [s_load, s_imp, s_zr, s_hh]
            offs = _TUNE["offs_b"]
            sts = {}
            for r in range(NCH + max(offs)):
                if r > 0:
                    for j in pending_w.pop(r - 1, ()):
                        nc.sync.dma_start(out=w_sb[:, j], in_=w_r[:, j])
                for s, off in zip(stages, offs):
                    j = r - off
                    if 0 <= j < NCH:
                        if s is s_load:
                            sts[j] = s(j)
                        else:
                            s(sts[j])
            sts.clear()

            if reps > 1:
                loop_ctx.__exit__(None, None, None)

    nc.compile()
    return nc, need


class _Runner:
    """Cached jitted shard_map executor (mirrors bass2jax.run_bass_via_pjrt).

    Inputs are passed as FULL concatenated arrays (shape[0] = 8x the
    per-core shard), which shard_map slices along axis 0 — a per-core
    shard list concatenated on axis 0 is just the original full tensor.
    """

    def __init__(self, nc):
        import jax
        from concourse import bass2jax, mybir

        bass2jax.install_neuronx_cc_hook()

        partition_name = (nc.partition_id_tensor.name
                          if nc.partition_id_tensor else None)
        in_names, out_names, out_avals = [], [], []
        for alloc in nc.m.functions[0].allocations:
            if not isinstance(alloc, mybir.MemoryLocationSet):
                continue
            name = alloc.memorylocations[0].name
            if alloc.kind == "ExternalInput":
                if name != partition_name:
                    in_names.append(name)
            elif alloc.kind == "ExternalOutput":
                out_names.append(name)
                out_avals.append(jax.core.ShapedArray(
                    tuple(alloc.tensor_shape), mybir.dt.np(alloc.dtype)))
        n_params = len(in_names)
        n_outs = len(out_names)
        all_names = tuple(in_names) + tuple(out_names)
        if partition_name is not None:
            all_names = all_names + (partition_name,)

        def _body(*args):
            operands = list(args)
            if partition_name is not None:
                operands.append(bass2jax.partition_id_tensor())
            outs = bass2jax._bass_exec_p.bind(
                *operands,
                out_avals=tuple(out_avals),
                in_names=all_names,
                out_names=tuple(out_names),
                lowering_input_output_aliases=(),
                sim_require_finite=True,
                sim_require_nnan=True,
                nc=nc,
            )
            return tuple(outs)

        devices = jax.devices()[:N_CORES]
        self.mesh = bass2jax.Mesh(np.asarray(devices), ("core",))
        self.spec = bass2jax.PartitionSpec("core")
        rep = bass2jax.PartitionSpec()
        # inp/hp are batch-sharded; everything else is replicated.
        self.in_specs = tuple(
            self.spec if name in ("inp", "hp") else rep for name in in_names)
        self.sharded = jax.jit(
            bass2jax.shard_map(
                _body, mesh=self.mesh,
                in_specs=self.in_specs + (self.spec,) * n_outs,
                out_specs=(self.spec,) * n_outs,
                check_rep=False),
            donate_argnums=tuple(range(n_params, n_params + n_outs)),
            keep_unused=True,
        )
        self.in_names = in_names
        self.out_names = out_names
        self.out_avals = out_avals

    def zeros(self):
        return [np.zeros((N_CORES * a.shape[0], *a.shape[1:]), a.dtype)
                for a in self.out_avals]

    def __call__(self, full_arrays):
        """full_arrays: dict name -> full (8x-shard-concat) np array."""
        concat_in = [full_arrays[name] for name in self.in_names]
        out_arrs = self.sharded(*concat_in, *self.zeros())
        return np.asarray(out_arrs[0])


def _get_runner(flags, reps=1):
    with _lock:
        key = (flags, reps)
        if key not in _cache:
            nc, need = _build_nc(flags, reps)
            _cache[key] = (_Runner(nc), need)
        return _cache[key]


def _host_arrays(inputs):
    """Host-side value prep shared by kernel() and the test harnesses.
    Returns (flags, full_arrays dict keyed by dram-tensor name) — extras
    included unconditionally; callers filter by the program's `need`."""
    inp = np.ascontiguousarray(inputs["inputs"], dtype=np.float32)
    hp = np.ascontiguousarray(inputs["h_prev"], dtype=np.float32)
    W = np.stack([np.asarray(inputs[k], dtype=np.float32)
                  for k in ("W_z", "U_z", "W_r", "U_r", "W_h", "U_h")])
    # The r-gate is computed as rt = tanh(r_pre/2) with rh' = hd + rt*hd
    # = 2*(r*hd); the missing 0.5 of sigmoid(a) = 0.5 + 0.5*tanh(a/2) is
    # folded into U_h here (rh' only ever feeds rh' @ U_h).
    W[5] *= 0.5
    gxr = np.maximum(np.asarray(inputs["gamma_x_decay"], np.float32), 0.0)
    ghr = np.maximum(np.asarray(inputs["gamma_h_decay"], np.float32), 0.0)
    mu = np.asarray(inputs["mean_imputation"], np.float32)
    bz = np.asarray(inputs["b_z"], np.float32)
    br = np.asarray(inputs["b_r"], np.float32)
    bh = np.asarray(inputs["b_h"], np.float32)

    gx_one = bool(np.all(gxr == 1.0))
    gh_one = bool(np.all(ghr == 1.0))
    shared = bool(np.array_equal(gxr, ghr))
    flags = (shared, gx_one, gh_one,
             not mu.any(), not bz.any(), not br.any(), not bh.any())

    full = {"inp": _to_bf16(inp), "hp": _to_bf16(hp), "w": _to_bf16(W)}
    if _TUNE.get("layout", "A") == "B":
        # B contract: decays are negated fp32 ACT scales; z/r biases are
        # pre-scaled by 0.5 (fused into tanh(a/2 + b/2)).
        full["gxr"] = -gxr
        full["ghr"] = -ghr
        full["mu"] = mu
        full["bz"] = 0.5 * bz
        full["br"] = 0.5 * br
        full["bh"] = bh
    else:
        full["gxr"] = _to_bf16(gxr)
        full["ghr"] = _to_bf16(ghr)
        full["mu"] = _to_bf16(mu)
        full["bz"] = bz
        full["br"] = br
        full["bh"] = bh
    return flags, full


def _prep(inputs):
    """Returns (runner, full_arrays dict keyed by dram-tensor name)."""
    flags, full = _host_arrays(inputs)
    run, need = _get_runner(flags)
    return run, {k: v for k, v in full.items()
                 if k in ("inp", "hp", "w") or k in need}


def kernel(**inputs) -> np.ndarray:
    run, full = _prep(inputs)
    return run(full)


def _timed_calls(run, full, iters):
    import time

    import jax
    from jax.sharding import NamedSharding

    out_sh = NamedSharding(run.mesh, run.spec)
    dev_in = [
        jax.device_put(full[name], NamedSharding(run.mesh, spec))
        for name, spec in zip(run.in_names, run.in_specs)
    ]
    zero_sets = [[jax.device_put(z, out_sh) for z in run.zeros()]
                 for _ in range(iters)]
    jax.block_until_ready(dev_in)
    jax.block_until_ready(zero_sets)
    jax.block_until_ready(run.sharded(*dev_in, *run.zeros()))  # warm-up
    times = []
    for i in range(iters):
        t0 = time.perf_counter_ns()
        out = run.sharded(*dev_in, *zero_sets[i])
        jax.block_until_ready(out)
        times.append(time.perf_counter_ns() - t0)
    times.sort()
    return times


def bench_device(inputs, iters: int = 14, reps_lo: int = 8, reps_hi: int = 64):
    """Estimate on-device per-pass execution time via loop-delta timing:
    two looped variants of the kernel (hardware For-loop in one NEFF, same
    code shape) amortize the large axon dispatch overhead; per-pass time =
    (median(T_hi) - median(T_lo)) / (reps_hi - reps_lo). Medians because
    the dispatch noise has heavy two-sided outliers. The loop back-edge is
    a full barrier, so this measures a complete pass including pipeline
    fill/drain — a slight over-estimate of the streamed single-shot time."""
    run1, full = _prep(inputs)
    flags_key = next(k for k in _cache if _cache[k][0] is run1)[0]
    run_lo, _ = _get_runner(flags_key, reps=reps_lo)
    run_hi, _ = _get_runner(flags_key, reps=reps_hi)

    t_lo = _timed_calls(run_lo, full, iters)
    t_hi = _timed_calls(run_hi, full, iters)
    med_lo = t_lo[len(t_lo) // 2]
    med_hi = t_hi[len(t_hi) // 2]
    per_pass = (med_hi - med_lo) / (reps_hi - reps_lo)
    return {
        "per_pass_ns": int(per_pass),
        "t_lo_med_ns": med_lo,
        "t_hi_med_ns": med_hi,
        "t_lo_all": t_lo,
        "t_hi_all": t_hi,
    }


# revision 68
# speedup vs baseline: 1.0657x; 1.0657x over previous
"""GRU-D cell on 8 Trainium2 NeuronCores (Bass/Tile SPMD kernel).

Data-parallel: the batch dim (16384) is sharded 8 x 2048 across cores;
the six 512x512 weight matrices are replicated. Per core, the 2048-row
shard is processed as 16 tiles of 128 rows (batch on partitions):

  gamma   = exp(-relu(gamma_decay) * delta_t)          (ACT, fused scale)
  x_dec   = x * (m + gx - m*gx)  [+ mu * (1 - q)]      (DVE, bf16)
  h_dec   = gh * h_prev                                (DVE)
  z/r/hh  : PE matmuls, contraction dim on partitions via PE transposes
            of x_dec / h_dec / (r*h_dec); weights replicated in SBUF bf16
  h_new   = h_dec + z * (tanh(hh_pre) - h_dec)         (DVE, fp32 out)

Inputs are cast to bf16 on the host (round-to-nearest) to halve HBM
traffic and let the PE run at full bf16 rate; accumulation stays fp32 in
PSUM. Zero biases / zero mean-imputation / all-ones decay vectors (the
values this problem ships) are detected at runtime and the matching
pre-compiled specialization is used; non-trivial values fall back to a
general variant built from the same emitter.
"""

import os
import threading
from contextlib import ExitStack

import ml_dtypes
import numpy as np

F = 512
P = 128
N_CORES = 8
B = 16384
BC = B // N_CORES          # rows per core
NT = BC // P               # 128-row tiles per core
KC = F // P                # contraction chunks

BF16 = ml_dtypes.bfloat16

_lock = threading.Lock()
_cache = {}

# Tunables (settled via timeline-sim scans + HW loop-delta timing).
_TUNE = {
    "hd_eng": "vector",
    "rh_eng": "vector",
    "xdT_eng": "vector",
    "hdT_eng": "scalar",
    "rhT_eng": "vector",
    "wk_bufs": 5,
    "io_bufs": 3,
    "pt_bufs": 4,
    "pg_bufs": 4,
    "offs": (0, 1, 2, 3, 4),
    "zaff_eng": "gpsimd",
    "d_eng": "vector",
    "zd_eng": "vector",
    "w_split": True,
    "warmup": 24,
    # layout B knobs
    "layout": "A",
    "wkb_bufs": 3,
    "iob_bufs": 3,
    "pgb_bufs": 6,
    "ptb_bufs": 2,
    "offs_b": (0, 1, 2, 3),
    "hnc_eng": "scalar",
}


def _HD_ENG(nc):
    return getattr(nc, _TUNE["hd_eng"])


def _RH_ENG(nc):
    return getattr(nc, _TUNE["rh_eng"])


def _copy(nc, eng, out, in_):
    if eng == "scalar":
        nc.scalar.copy(out, in_)
    else:
        getattr(nc, eng).tensor_copy(out, in_)


def _to_bf16(a: np.ndarray) -> np.ndarray:
    """Fast fp32 -> bf16 with round-to-nearest (ties toward +1 ulp)."""
    a = np.ascontiguousarray(a, dtype=np.float32)
    u = a.view(np.uint32)
    r = (u >> 16) & np.uint32(1)
    out = ((u + np.uint32(0x7FFF) + r) >> 16).astype(np.uint16)
    return out.view(BF16).reshape(a.shape)


def _build_nc(flags, reps=1):
    if _TUNE.get("layout", "A") == "B":
        return _build_nc_b(flags, reps)
    return _build_nc_a(flags, reps)


def _build_nc_a(flags, reps=1):
    """Emit + compile the Bass program for one specialization.

    flags = (shared_gamma, gx_one, gh_one, mu_zero, bz_zero, br_zero, bh_zero)
    reps > 1 unrolls the whole pass multiple times in one NEFF — used only
    for device-time measurement (loop-delta timing amortizes dispatch).
    Returns (nc, in_names_needed).
    """
    import concourse.bass as bass
    import concourse.tile as tile
    from concourse import bacc, mybir
    from concourse.bass import ts
    from concourse.masks import make_identity

    shared_gamma, gx_one, gh_one, mu_zero, bz_zero, br_zero, bh_zero = flags
    BD = mybir.dt.bfloat16
    FD = mybir.dt.float32
    AF = mybir.ActivationFunctionType

    nc = bacc.Bacc("TRN2", target_bir_lowering=False, debug=False,
                   num_devices=N_CORES)

    inp = nc.dram_tensor("inp", [BC, 3 * F], BD, kind="ExternalInput").ap()
    hp = nc.dram_tensor("hp", [BC, F], BD, kind="ExternalInput").ap()
    w = nc.dram_tensor("w", [6, F, F], BD, kind="ExternalInput").ap()
    need = ["inp", "hp", "w"]
    gxr = ghr = mu = bz = br = bh = None
    if not gx_one:
        gxr = nc.dram_tensor("gxr", [F], BD, kind="ExternalInput").ap()
        need.append("gxr")
    if not shared_gamma and not gh_one:
        ghr = nc.dram_tensor("ghr", [F], BD, kind="ExternalInput").ap()
        need.append("ghr")
    if not mu_zero:
        mu = nc.dram_tensor("mu", [F], BD, kind="ExternalInput").ap()
        need.append("mu")
    if not bz_zero:
        bz = nc.dram_tensor("bz", [F], FD, kind="ExternalInput").ap()
        need.append("bz")
    if not br_zero:
        br = nc.dram_tensor("br", [F], FD, kind="ExternalInput").ap()
        need.append("br")
    if not bh_zero:
        bh = nc.dram_tensor("bh", [F], FD, kind="ExternalInput").ap()
        need.append("bh")
    out = nc.dram_tensor("h_new", [BC, F], FD, kind="ExternalOutput").ap()

    inp_r = inp.rearrange("(n p) c -> n p c", p=P)
    hp_r = hp.rearrange("(n p) c -> n p c", p=P)
    out_r = out.rearrange("(n p) c -> n p c", p=P)
    w_r = w.rearrange("w (k p) u -> p w k u", p=P)

    def bcast(pool, dram_ap, dt, name):
        t = pool.tile([P, F], dt, name=name)
        src = bass.AP(tensor=dram_ap.tensor, offset=dram_ap.offset,
                      ap=[[0, P]] + [list(d) for d in dram_ap.ap])
        nc.gpsimd.dma_start(out=t, in_=src)
        return t

    with tile.TileContext(nc) as tc:
        with ExitStack() as ctx:
            consts = ctx.enter_context(tc.tile_pool(name="consts", bufs=1))
            wk = ctx.enter_context(
                tc.tile_pool(name="wk", bufs=_TUNE["wk_bufs"]))
            io_pool = ctx.enter_context(
                tc.tile_pool(name="io", bufs=_TUNE["io_bufs"]))
            pt = ctx.enter_context(
                tc.tile_pool(name="pt", bufs=_TUNE["pt_bufs"], space="PSUM"))
            pg = ctx.enter_context(
                tc.tile_pool(name="pg", bufs=_TUNE["pg_bufs"], space="PSUM"))

            ident = consts.tile([P, P], BD)
            make_identity(nc, ident)
            w_sb = consts.tile([P, 6, KC, F], BD)
            pending_w = {}
            if _TUNE["w_split"]:
                # Weight loads are emitted into the early pipeline rounds
                # (after the first input tiles) so the first transposes and
                # matmuls aren't gated on the full 3MB weight load.
                pending_w = {0: [0, 1], 1: [2, 3], 2: [4, 5]}
            else:
                nc.sync.dma_start(out=w_sb, in_=w_r)
            if _TUNE["warmup"]:
                # The PE clock ramps with sustained use (full speed only
                # after ~3us busy). Run throwaway transposes on the identity
                # during the initial DMA fill so the first real matmuls start
                # at full clock. (Reuses a pt-pool slot; WAW on one tile
                # serializes them on the PE, which is exactly the point.)
                wu_p = pt.tile([P, P], BD, tag="tp", name="wu_p")
                for _ in range(_TUNE["warmup"]):
                    nc.tensor.transpose(wu_p, ident, ident)
            gxb = ghb = mub = bzb = brb = bhb = None
            if gxr is not None:
                gxb = bcast(consts, gxr, BD, "gxb")
            if ghr is not None:
                ghb = bcast(consts, ghr, BD, "ghb")
            if mu is not None:
                mub = bcast(consts, mu, BD, "mub")
            if bz is not None:
                bzb = bcast(consts, bz, FD, "bzb")
            if br is not None:
                brb = bcast(consts, br, FD, "brb")
            if bh is not None:
                bhb = bcast(consts, bh, FD, "bhb")

            if reps > 1:
                loop_ctx = tc.For_i(0, reps, 1)
                loop_ctx.__enter__()

            def stage_a1(i):
                """DMA-in, imputation/decay, xd/hd transposes."""
                it = io_pool.tile([P, 3 * F], BD, tag="it")
                if i == 0:
                    # delta_t slice first: it gates the exp that heads the
                    # whole dependency chain of the first tile.
                    nc.sync.dma_start(out=it[:, 2 * F:], in_=inp_r[i][:, 2 * F:])
                    nc.sync.dma_start(out=it[:, :2 * F], in_=inp_r[i][:, :2 * F])
                else:
                    nc.sync.dma_start(out=it, in_=inp_r[i])
                ht = io_pool.tile([P, F], BD, tag="ht")
                nc.sync.dma_start(out=ht, in_=hp_r[i])
                x_ = it[:, 0:F]
                m_ = it[:, F:2 * F]
                dt_ = it[:, 2 * F:3 * F]

                # gamma_x = exp(-relu(gx) * delta_t)
                g = wk.tile([P, F], BD, tag="g")
                if gx_one:
                    if i == 0:
                        nc.scalar.activation(g[:, :F // 2], dt_[:, :F // 2],
                                             AF.Exp, scale=-1.0)
                        nc.scalar.activation(g[:, F // 2:], dt_[:, F // 2:],
                                             AF.Exp, scale=-1.0)
                    else:
                        nc.scalar.activation(g, dt_, AF.Exp, scale=-1.0)
                else:
                    ga = wk.tile([P, F], BD, tag="ga")
                    nc.vector.tensor_mul(ga, dt_, gxb)
                    nc.scalar.activation(g, ga, AF.Exp, scale=-1.0)
                if shared_gamma:
                    gH = g
                elif gh_one:
                    gH = wk.tile([P, F], BD, tag="gH")
                    nc.scalar.activation(gH, dt_, AF.Exp, scale=-1.0)
                else:
                    gha = wk.tile([P, F], BD, tag="gha")
                    nc.vector.tensor_mul(gha, dt_, ghb)
                    gH = wk.tile([P, F], BD, tag="gH")
                    nc.scalar.activation(gH, gha, AF.Exp, scale=-1.0)

                # x_decayed = x*q (+ mu*(1-q)), q = m + g - m*g
                # For the first tile, run the chain in column halves so the
                # first PE transposes unblock (via subtile deps) ~1us sooner.
                t1 = wk.tile([P, F], BD, tag="t1")
                q = wk.tile([P, F], BD, tag="q")
                q2 = wk.tile([P, F], BD, tag="q2")
                xd = wk.tile([P, F], BD, tag="xd")
                halves = ([slice(0, F // 2), slice(F // 2, F)]
                          if i == 0 else [slice(0, F)])
                for sl in halves:
                    nc.vector.tensor_mul(t1[:, sl], m_[:, sl], g[:, sl])
                    nc.vector.tensor_add(q[:, sl], m_[:, sl], g[:, sl])
                    nc.vector.tensor_sub(q2[:, sl], q[:, sl], t1[:, sl])
                    nc.vector.tensor_mul(xd[:, sl], x_[:, sl], q2[:, sl])
                if not mu_zero:
                    omq = wk.tile([P, F], BD, tag="omq")
                    nc.vector.tensor_scalar(
                        omq, q2, -1.0, 1.0,
                        mybir.AluOpType.mult, mybir.AluOpType.add)
                    muq = wk.tile([P, F], BD, tag="muq")
                    nc.vector.tensor_mul(muq, omq, mub)
                    xd2 = wk.tile([P, F], BD, tag="xd2")
                    nc.vector.tensor_add(xd2, xd, muq)
                    xd = xd2
                hd = wk.tile([P, F], BD, tag="hd")
                _HD_ENG(nc).tensor_mul(hd, gH, ht)
                return dict(xd=xd, hd=hd, i=i)

            def stage_tr(st):
                """PE transposes of xd/hd: [128-batch, 512-f] -> [512-f, 128-b]."""
                xd, hd = st["xd"], st["hd"]
                xdT_p = pt.tile([P, F], BD, tag="tp")
                for k in range(KC):
                    nc.tensor.transpose(xdT_p[:, ts(k, P)], xd[:, ts(k, P)], ident)
                xdT = wk.tile([P, F], BD, tag="xdT")
                _copy(nc, _TUNE["xdT_eng"], xdT, xdT_p)
                hdT_p = pt.tile([P, F], BD, tag="tp")
                for k in range(KC):
                    nc.tensor.transpose(hdT_p[:, ts(k, P)], hd[:, ts(k, P)], ident)
                hdT = wk.tile([P, F], BD, tag="hdT")
                _copy(nc, _TUNE["hdT_eng"], hdT, hdT_p)
                st.update(xdT=xdT, hdT=hdT)
                return st

            def stage_zr(st):
                """z/r gate matmuls, sigmoids, rh."""
                xdT, hdT, hd = st["xdT"], st["hdT"], st["hd"]
                zp = pg.tile([P, F], FD, tag="gp")
                for k in range(KC):
                    nc.tensor.matmul(zp, xdT[:, ts(k, P)], w_sb[:, 0, k, :],
                                     start=(k == 0), stop=False)
                for k in range(KC):
                    nc.tensor.matmul(zp, hdT[:, ts(k, P)], w_sb[:, 1, k, :],
                                     start=False, stop=(k == KC - 1))
                rp = pg.tile([P, F], FD, tag="gp")
                for k in range(KC):
                    nc.tensor.matmul(rp, xdT[:, ts(k, P)], w_sb[:, 2, k, :],
                                     start=(k == 0), stop=False)
                for k in range(KC):
                    nc.tensor.matmul(rp, hdT[:, ts(k, P)], w_sb[:, 3, k, :],
                                     start=False, stop=(k == KC - 1))
                # sigmoid(a) = 0.5 + 0.5*tanh(a/2): keeps every ACT func in
                # the single `exp_and_others` table (Exp/Tanh/Copy) — a
                # Sigmoid would force ~1.3us act-table reloads per switch.
                zt = wk.tile([P, F], BD, tag="zt")
                if bz_zero:
                    nc.scalar.activation(zt, zp, AF.Tanh, scale=0.5)
                else:
                    zb = wk.tile([P, F], FD, tag="zb")
                    nc.vector.tensor_add(zb, zp, bzb)
                    nc.scalar.activation(zt, zb, AF.Tanh, scale=0.5)
                z = wk.tile([P, F], BD, tag="z")
                getattr(nc, _TUNE["zaff_eng"]).tensor_scalar(
                    z, zt, 0.5, 0.5,
                    mybir.AluOpType.mult, mybir.AluOpType.add)
                rt = wk.tile([P, F], BD, tag="rt")
                if br_zero:
                    nc.scalar.activation(rt, rp, AF.Tanh, scale=0.5)
                else:
                    rb = wk.tile([P, F], FD, tag="rb")
                    nc.vector.tensor_add(rb, rp, brb)
                    nc.scalar.activation(rt, rb, AF.Tanh, scale=0.5)
                # r = 0.5 + 0.5*rt, and rh = r*hd feeds only (rh @ U_h).
                # The 0.5 factor is folded into U_h host-side, so
                # rh' = hd + rt*hd  (saves the affine op on r).
                v = wk.tile([P, F], BD, tag="v")
                nc.vector.tensor_mul(v, rt, hd)
                rh = wk.tile([P, F], BD, tag="rh")
                _RH_ENG(nc).tensor_add(rh, hd, v)
                st.update(rh=rh, z=z)
                return st

            def stage_rt(st):
                """rh transpose."""
                rh = st["rh"]
                rhT_p = pt.tile([P, F], BD, tag="tp")
                for k in range(KC):
                    nc.tensor.transpose(rhT_p[:, ts(k, P)], rh[:, ts(k, P)], ident)
                rhT = wk.tile([P, F], BD, tag="rhT")
                _copy(nc, _TUNE["rhT_eng"], rhT, rhT_p)
                st.update(rhT=rhT)
                return st

            def stage_b(st):
                """hh gate matmuls, tanh, blend, DMA-out.

                The last tile runs in two N=256 column halves so the
                closing tanh->blend->store chain (the kernel's drain tail)
                overlaps the second half's matmuls."""
                xdT, rhT, hd, z, i = (st["xdT"], st["rhT"], st["hd"],
                                      st["z"], st["i"])
                hh = wk.tile([P, F], BD, tag="hh")
                d = wk.tile([P, F], BD, tag="d")
                zd = wk.tile([P, F], BD, tag="zd")
                hn = wk.tile([P, F], FD, tag="hn")
                halves = ([slice(0, F // 2), slice(F // 2, F)]
                          if i == NT - 1 else [slice(0, F)])
                for sl in halves:
                    hpp = pg.tile([P, sl.stop - sl.start], FD, tag="gp",
                                  name="hpp")
                    for k in range(KC):
                        nc.tensor.matmul(hpp, xdT[:, ts(k, P)],
                                         w_sb[:, 4, k, sl],
                                         start=(k == 0), stop=False)
                    for k in range(KC):
                        nc.tensor.matmul(hpp, rhT[:, ts(k, P)],
                                         w_sb[:, 5, k, sl],
                                         start=False, stop=(k == KC - 1))
                    if bh_zero:
                        nc.scalar.activation(hh[:, sl], hpp, AF.Tanh)
                    else:
                        hb = wk.tile([P, F], FD, tag="hb")
                        nc.vector.tensor_add(hb[:, sl], hpp, bhb[:, sl])
                        nc.scalar.activation(hh[:, sl], hb[:, sl], AF.Tanh)

                    # h_new = hd + z*(hh - hd)
                    getattr(nc, _TUNE["d_eng"]).tensor_sub(
                        d[:, sl], hh[:, sl], hd[:, sl])
                    getattr(nc, _TUNE["zd_eng"]).tensor_mul(
                        zd[:, sl], z[:, sl], d[:, sl])
                    nc.vector.tensor_add(hn[:, sl], hd[:, sl], zd[:, sl])
                    nc.sync.dma_start(out=out_r[i][:, sl], in_=hn[:, sl])

            stages = [stage_a1, stage_tr, stage_zr, stage_rt, stage_b]
            offs = _TUNE["offs"]
            sts = {}
            for r in range(NT + max(offs)):
                for s, off in zip(stages, offs):
                    j = r - off
                    if 0 <= j < NT:
                        if s is stage_a1:
                            sts[j] = s(j)
                        else:
                            s(sts[j])
                for j in pending_w.pop(r, ()):
                    nc.sync.dma_start(out=w_sb[:, j], in_=w_r[:, j])
            sts.clear()

            if reps > 1:
                loop_ctx.__exit__(None, None, None)

    nc.compile()
    return nc, need


def _build_nc_b(flags, reps=1):
    """Layout B: feature-major compute via DMA-transposed input loads.

    Inputs land in SBUF already transposed ([feature-part, batch-free],
    512-row batch chunks), so the imputation/decay math, gates and blend
    all run feature-major: no PE input transposes, no PSUM->SBUF copies
    for matmul operands, and biases/decay vectors fuse into ACT as
    per-partition scalars. Only the final h_new needs PE transposes back
    to batch-major. Weights are the stationary matmul operand here.
    """
    import concourse.bass as bass
    import concourse.tile as tile
    from concourse import bacc, mybir
    from concourse.bass import ts
    from concourse.masks import make_identity

    shared_gamma, gx_one, gh_one, mu_zero, bz_zero, br_zero, bh_zero = flags
    BD = mybir.dt.bfloat16
    FD = mybir.dt.float32
    AF = mybir.ActivationFunctionType
    CW = 512               # batch columns per chunk
    NCH = BC // CW         # chunks per core (4)

    nc = bacc.Bacc("TRN2", target_bir_lowering=False, debug=False,
                   num_devices=N_CORES)

    inp = nc.dram_tensor("inp", [BC, 3 * F], BD, kind="ExternalInput").ap()
    hp = nc.dram_tensor("hp", [BC, F], BD, kind="ExternalInput").ap()
    w = nc.dram_tensor("w", [6, F, F], BD, kind="ExternalInput").ap()
    need = ["inp", "hp", "w"]
    gxr = ghr = mu = bz = br = bh = None
    if not gx_one:
        # host passes NEGATED relu'd decay (used as ACT scale)
        gxr = nc.dram_tensor("gxr", [F], FD, kind="ExternalInput").ap()
        need.append("gxr")
    if not shared_gamma and not gh_one:
        ghr = nc.dram_tensor("ghr", [F], FD, kind="ExternalInput").ap()
        need.append("ghr")
    if not mu_zero:
        mu = nc.dram_tensor("mu", [F], FD, kind="ExternalInput").ap()
        need.append("mu")
    if not bz_zero:
        bz = nc.dram_tensor("bz", [F], FD, kind="ExternalInput").ap()
        need.append("bz")
    if not br_zero:
        br = nc.dram_tensor("br", [F], FD, kind="ExternalInput").ap()
        need.append("br")
    if not bh_zero:
        bh = nc.dram_tensor("bh", [F], FD, kind="ExternalInput").ap()
        need.append("bh")
    out = nc.dram_tensor("h_new", [BC, F], FD, kind="ExternalOutput").ap()

    w_r = w.rearrange("w (k p) u -> p w k u", p=P)

    def fmaj(pool, dram_ap, name):
        # [F] vector -> [128, KC] feature-major ([p, c] = v[c*128+p])
        t = pool.tile([P, KC], FD, name=name)
        nc.gpsimd.dma_start(out=t, in_=dram_ap.rearrange("(c p) -> p c", p=P))
        return t

    with tile.TileContext(nc) as tc:
        with ExitStack() as ctx:
            consts = ctx.enter_context(tc.tile_pool(name="consts", bufs=1))
            wk = ctx.enter_context(
                tc.tile_pool(name="wk", bufs=_TUNE["wkb_bufs"]))
            io_pool = ctx.enter_context(
                tc.tile_pool(name="io", bufs=_TUNE["iob_bufs"]))
            pg = ctx.enter_context(
                tc.tile_pool(name="pg", bufs=_TUNE["pgb_bufs"], space="PSUM"))
            pt = ctx.enter_context(
                tc.tile_pool(name="pt", bufs=_TUNE["ptb_bufs"], space="PSUM"))

            ident = consts.tile([P, P], BD)
            make_identity(nc, ident)
            w_sb = consts.tile([P, 6, KC, F], BD)
            pending_w = {0: [0, 1, 2, 3], 1: [4, 5]}

            gxb = ghb = mub = nmub = bzb = brb = bhb = None
            if gxr is not None:
                gxb = fmaj(consts, gxr, "gxb")
            if ghr is not None:
                ghb = fmaj(consts, ghr, "ghb")
            if mu is not None:
                mub = fmaj(consts, mu, "mub")
                nmub = consts.tile([P, KC], FD, name="nmub")
                nc.vector.tensor_scalar(nmub, mub, -1.0, None,
                                        mybir.AluOpType.mult)
            if bz is not None:
                bzb = fmaj(consts, bz, "bzb")   # host pre-scaled by 0.5
            if br is not None:
                brb = fmaj(consts, br, "brb")   # host pre-scaled by 0.5
            if bh is not None:
                bhb = fmaj(consts, bh, "bhb")

            def s_load(c):
                """DMA-transpose x/m/dt/h for batch rows [c*CW, (c+1)*CW)."""
                rows = slice(c * CW, (c + 1) * CW)
                xT = io_pool.tile([P, KC, CW], BD, tag="xT")
                mT = io_pool.tile([P, KC, CW], BD, tag="mT")
                dT = io_pool.tile([P, KC, CW], BD, tag="dT")
                hT = io_pool.tile([P, KC, CW], BD, tag="hT")
                # delta_t first (the exp is the first consumer), then m, x, h
                for f in range(KC):
                    nc.sync.dma_start(
                        out=dT[:, f, :],
                        in_=inp[rows, 2 * F + f * P:2 * F + (f + 1) * P],
                        transpose=True)
                for f in range(KC):
                    nc.sync.dma_start(out=mT[:, f, :],
                                      in_=inp[rows, F + f * P:F + (f + 1) * P],
                                      transpose=True)
                for f in range(KC):
                    nc.sync.dma_start(out=xT[:, f, :],
                                      in_=inp[rows, ts(f, P)], transpose=True)
                for f in range(KC):
                    nc.sync.dma_start(out=hT[:, f, :],
                                      in_=hp[rows, ts(f, P)], transpose=True)
                return dict(xT=xT, mT=mT, dT=dT, hT=hT, c=c)

            def s_imp(st):
                """Imputation + decay, feature-major, mostly in place:
                xT -> x_decayed, hT -> h_decayed, mT/dT scratch."""
                xT, mT, dT, hT = st["xT"], st["mT"], st["dT"], st["hT"]
                g = wk.tile([P, KC, CW], BD, tag="g")
                if gx_one:
                    nc.scalar.activation(g, dT, AF.Exp, scale=-1.0)
                else:
                    for f in range(KC):
                        nc.scalar.activation(g[:, f, :], dT[:, f, :], AF.Exp,
                                             scale=gxb[:, f:f + 1])
                if shared_gamma:
                    gH = g
                elif gh_one:
                    gH = wk.tile([P, KC, CW], BD, tag="gH")
                    nc.scalar.activation(gH, dT, AF.Exp, scale=-1.0)
                else:
                    gH = wk.tile([P, KC, CW], BD, tag="gH")
                    for f in range(KC):
                        nc.scalar.activation(gH[:, f, :], dT[:, f, :], AF.Exp,
                                             scale=ghb[:, f:f + 1])
                # q = m + g - m*g  (dT <- m*g, mT <- q)
                nc.vector.tensor_mul(dT, mT, g)
                nc.vector.tensor_add(mT, mT, g)
                nc.vector.tensor_sub(mT, mT, dT)
                # xd = x*q (+ mu*(1-q))
                nc.vector.tensor_mul(xT, xT, mT)
                if not mu_zero:
                    for f in range(KC):
                        nc.vector.tensor_scalar(
                            dT[:, f, :], mT[:, f, :],
                            nmub[:, f:f + 1], mub[:, f:f + 1],
                            mybir.AluOpType.mult, mybir.AluOpType.add)
                    nc.vector.tensor_add(xT, xT, dT)
                # hd = gH * h
                nc.vector.tensor_mul(hT, gH, hT)
                st["g"] = g
                return st

            def s_zr(st):
                """z and r gates + rh (in place on rt)."""
                xT, hT = st["xT"], st["hT"]
                zt = wk.tile([P, KC, CW], BD, tag="zt")
                for u in range(KC):
                    zp = pg.tile([P, CW], FD, tag="gp")
                    for f in range(KC):
                        nc.tensor.matmul(zp, w_sb[:, 0, f, ts(u, P)],
                                         xT[:, f, :],
                                         start=(f == 0), stop=False)
                    for f in range(KC):
                        nc.tensor.matmul(zp, w_sb[:, 1, f, ts(u, P)],
                                         hT[:, f, :],
                                         start=False, stop=(f == KC - 1))
                    if bz_zero:
                        nc.scalar.activation(zt[:, u, :], zp, AF.Tanh,
                                             scale=0.5)
                    else:
                        nc.scalar.activation(zt[:, u, :], zp, AF.Tanh,
                                             scale=0.5, bias=bzb[:, u:u + 1])
                z = wk.tile([P, KC, CW], BD, tag="z")
                getattr(nc, _TUNE["zaff_eng"]).tensor_scalar(
                    z, zt, 0.5, 0.5,
                    mybir.AluOpType.mult, mybir.AluOpType.add)
                rt = wk.tile([P, KC, CW], BD, tag="rt")
                for u in range(KC):
                    rp = pg.tile([P, CW], FD, tag="gp")
                    for f in range(KC):
                        nc.tensor.matmul(rp, w_sb[:, 2, f, ts(u, P)],
                                         xT[:, f, :],
                                         start=(f == 0), stop=False)
                    for f in range(KC):
                        nc.tensor.matmul(rp, w_sb[:, 3, f, ts(u, P)],
                                         hT[:, f, :],
                                         start=False, stop=(f == KC - 1))
                    if br_zero:
                        nc.scalar.activation(rt[:, u, :], rp, AF.Tanh,
                                             scale=0.5)
                    else:
                        nc.scalar.activation(rt[:, u, :], rp, AF.Tanh,
                                             scale=0.5, bias=brb[:, u:u + 1])
                # rh' = hd + rt*hd  (U_h host-scaled by 0.5); in place on rt
                nc.vector.tensor_mul(rt, rt, hT)
                nc.vector.tensor_add(rt, rt, hT)
                st.update(z=z, rh=rt)
                return st

            def s_hh(st):
                """hh gate, blend (in place -> h_new^T), transpose, store."""
                xT, hT, rh, z, c = st["xT"], st["hT"], st["rh"], st["z"], st["c"]
                hhT = wk.tile([P, KC, CW], BD, tag="hhT")
                for u in range(KC):
                    hpp = pg.tile([P, CW], FD, tag="gp")
                    for f in range(KC):
                        nc.tensor.matmul(hpp, w_sb[:, 4, f, ts(u, P)],
                                         xT[:, f, :],
                                         start=(f == 0), stop=False)
                    for f in range(KC):
                        nc.tensor.matmul(hpp, w_sb[:, 5, f, ts(u, P)],
                                         rh[:, f, :],
                                         start=False, stop=(f == KC - 1))
                    if bh_zero:
                        nc.scalar.activation(hhT[:, u, :], hpp, AF.Tanh)
                    else:
                        nc.scalar.activation(hhT[:, u, :], hpp, AF.Tanh,
                                             bias=bhb[:, u:u + 1])
                # h_new = hd + z*(hh - hd), in place on hhT
                nc.vector.tensor_sub(hhT, hhT, hT)
                nc.vector.tensor_mul(hhT, z, hhT)
                nc.vector.tensor_add(hhT, hT, hhT)
                # transpose back to batch-major and store
                for bs in range(CW // P):
                    hn_p = pt.tile([P, F], BD, tag="tp")
                    for u in range(KC):
                        nc.tensor.transpose(hn_p[:, ts(u, P)],
                                            hhT[:, u, ts(bs, P)], ident)
                    hn = wk.tile([P, F], FD, tag="hn")
                    _copy(nc, _TUNE["hnc_eng"], hn, hn_p)
                    nc.sync.dma_start(
                        out=out[c * CW + bs * P:c * CW + (bs + 1) * P, :],
                        in_=hn)

            if reps > 1:
                loop_ctx = tc.For_i(0, reps, 1)
                loop_ctx.__enter__()

            stages = [s_load, s_imp, s_zr, s_hh]
            offs = _TUNE["offs_b"]
            sts = {}
            for r in range(NCH + max(offs)):
                if r > 0:
                    for j in pending_w.pop(r - 1, ()):
                        nc.sync.dma_start(out=w_sb[:, j], in_=w_r[:, j])
                for s, off in zip(stages, offs):
                    j = r - off
                    if 0 <= j < NCH:
                        if s is s_load:
                            sts[j] = s(j)
                        else:
                            s(sts[j])
            sts.clear()

            if reps > 1:
                loop_ctx.__exit__(None, None, None)

    nc.compile()
    return nc, need


class _Runner:
    """Cached jitted shard_map executor (mirrors bass2jax.run_bass_via_pjrt).

    Inputs are passed as FULL concatenated arrays (shape[0] = 8x the
    per-core shard), which shard_map slices along axis 0 — a per-core
    shard list concatenated on axis 0 is just the original full tensor.
    """

    def __init__(self, nc):
        import jax
        from concourse import bass2jax, mybir

        bass2jax.install_neuronx_cc_hook()

        partition_name = (nc.partition_id_tensor.name
                          if nc.partition_id_tensor else None)
        in_names, out_names, out_avals = [], [], []
        for alloc in nc.m.functions[0].allocations:
            if not isinstance(alloc, mybir.MemoryLocationSet):
                continue
            name = alloc.memorylocations[0].name
            if alloc.kind == "ExternalInput":
                if name != partition_name:
                    in_names.append(name)
            elif alloc.kind == "ExternalOutput":
                out_names.append(name)
                out_avals.append(jax.core.ShapedArray(
                    tuple(alloc.tensor_shape), mybir.dt.np(alloc.dtype)))
        n_params = len(in_names)
        n_outs = len(out_names)
        all_names = tuple(in_names) + tuple(out_names)
        if partition_name is not None:
            all_names = all_names + (partition_name,)

        def _body(*args):
            operands = list(args)
            if partition_name is not None:
                operands.append(bass2jax.partition_id_tensor())
            outs = bass2jax._bass_exec_p.bind(
                *operands,
                out_avals=tuple(out_avals),
                in_names=all_names,
                out_names=tuple(out_names),
                lowering_input_output_aliases=(),
                sim_require_finite=True,
                sim_require_nnan=True,
                nc=nc,
            )
            return tuple(outs)

        devices = jax.devices()[:N_CORES]
        self.mesh = bass2jax.Mesh(np.asarray(devices), ("core",))
        self.spec = bass2jax.PartitionSpec("core")
        rep = bass2jax.PartitionSpec()
        # inp/hp are batch-sharded; everything else is replicated.
        self.in_specs = tuple(
            self.spec if name in ("inp", "hp") else rep for name in in_names)
        self.sharded = jax.jit(
            bass2jax.shard_map(
                _body, mesh=self.mesh,
                in_specs=self.in_specs + (self.spec,) * n_outs,
                out_specs=(self.spec,) * n_outs,
                check_rep=False),
            donate_argnums=tuple(range(n_params, n_params + n_outs)),
            keep_unused=True,
        )
        self.in_names = in_names
        self.out_names = out_names
        self.out_avals = out_avals

    def zeros(self):
        return [np.zeros((N_CORES * a.shape[0], *a.shape[1:]), a.dtype)
                for a in self.out_avals]

    def __call__(self, full_arrays):
        """full_arrays: dict name -> full (8x-shard-concat) np array."""
        concat_in = [full_arrays[name] for name in self.in_names]
        out_arrs = self.sharded(*concat_in, *self.zeros())
        return np.asarray(out_arrs[0])


def _get_runner(flags, reps=1):
    with _lock:
        key = (flags, reps)
        if key not in _cache:
            nc, need = _build_nc(flags, reps)
            _cache[key] = (_Runner(nc), need)
        return _cache[key]


def _host_arrays(inputs):
    """Host-side value prep shared by kernel() and the test harnesses.
    Returns (flags, full_arrays dict keyed by dram-tensor name) — extras
    included unconditionally; callers filter by the program's `need`."""
    inp = np.ascontiguousarray(inputs["inputs"], dtype=np.float32)
    hp = np.ascontiguousarray(inputs["h_prev"], dtype=np.float32)
    W = np.stack([np.asarray(inputs[k], dtype=np.float32)
                  for k in ("W_z", "U_z", "W_r", "U_r", "W_h", "U_h")])
    # The r-gate is computed as rt = tanh(r_pre/2) with rh' = hd + rt*hd
    # = 2*(r*hd); the missing 0.5 of sigmoid(a) = 0.5 + 0.5*tanh(a/2) is
    # folded into U_h here (rh' only ever feeds rh' @ U_h).
    W[5] *= 0.5
    gxr = np.maximum(np.asarray(inputs["gamma_x_decay"], np.float32), 0.0)
    ghr = np.maximum(np.asarray(inputs["gamma_h_decay"], np.float32), 0.0)
    mu = np.asarray(inputs["mean_imputation"], np.float32)
    bz = np.asarray(inputs["b_z"], np.float32)
    br = np.asarray(inputs["b_r"], np.float32)
    bh = np.asarray(inputs["b_h"], np.float32)

    gx_one = bool(np.all(gxr == 1.0))
    gh_one = bool(np.all(ghr == 1.0))
    shared = bool(np.array_equal(gxr, ghr))
    flags = (shared, gx_one, gh_one,
             not mu.any(), not bz.any(), not br.any(), not bh.any())

    full = {"inp": _to_bf16(inp), "hp": _to_bf16(hp), "w": _to_bf16(W)}
    if _TUNE.get("layout", "A") == "B":
        # B contract: decays are negated fp32 ACT scales; z/r biases are
        # pre-scaled by 0.5 (fused into tanh(a/2 + b/2)).
        full["gxr"] = -gxr
        full["ghr"] = -ghr
        full["mu"] = mu
        full["bz"] = 0.5 * bz
        full["br"] = 0.5 * br
        full["bh"] = bh
    else:
        full["gxr"] = _to_bf16(gxr)
        full["ghr"] = _to_bf16(ghr)
        full["mu"] = _to_bf16(mu)
        full["bz"] = bz
        full["br"] = br
        full["bh"] = bh
    return flags, full


def _prep(inputs):
    """Returns (runner, full_arrays dict keyed by dram-tensor name)."""
    flags, full = _host_arrays(inputs)
    run, need = _get_runner(flags)
    return run, {k: v for k, v in full.items()
                 if k in ("inp", "hp", "w") or k in need}


def kernel(**inputs) -> np.ndarray:
    run, full = _prep(inputs)
    return run(full)


def _timed_calls(run, full, iters):
    import time

    import jax
    from jax.sharding import NamedSharding

    out_sh = NamedSharding(run.mesh, run.spec)
    dev_in = [
        jax.device_put(full[name], NamedSharding(run.mesh, spec))
        for name, spec in zip(run.in_names, run.in_specs)
    ]
    zero_sets = [[jax.device_put(z, out_sh) for z in run.zeros()]
                 for _ in range(iters)]
    jax.block_until_ready(dev_in)
    jax.block_until_ready(zero_sets)
    jax.block_until_ready(run.sharded(*dev_in, *run.zeros()))  # warm-up
    times = []
    for i in range(iters):
        t0 = time.perf_counter_ns()
        out = run.sharded(*dev_in, *zero_sets[i])
        jax.block_until_ready(out)
        times.append(time.perf_counter_ns() - t0)
    times.sort()
    return times


def bench_device(inputs, iters: int = 14, reps_lo: int = 8, reps_hi: int = 64):
    """Estimate on-device per-pass execution time via loop-delta timing:
    two looped variants of the kernel (hardware For-loop in one NEFF, same
    code shape) amortize the large axon dispatch overhead; per-pass time =
    (median(T_hi) - median(T_lo)) / (reps_hi - reps_lo). Medians because
    the dispatch noise has heavy two-sided outliers. The loop back-edge is
    a full barrier, so this measures a complete pass including pipeline
    fill/drain — a slight over-estimate of the streamed single-shot time."""
    run1, full = _prep(inputs)
    flags_key = next(k for k in _cache if _cache[k][0] is run1)[0]
    run_lo, _ = _get_runner(flags_key, reps=reps_lo)
    run_hi, _ = _get_runner(flags_key, reps=reps_hi)

    t_lo = _timed_calls(run_lo, full, iters)
    t_hi = _timed_calls(run_hi, full, iters)
    med_lo = t_lo[len(t_lo) // 2]
    med_hi = t_hi[len(t_hi) // 2]
    per_pass = (med_hi - med_lo) / (reps_hi - reps_lo)
    return {
        "per_pass_ns": int(per_pass),
        "t_lo_med_ns": med_lo,
        "t_hi_med_ns": med_hi,
        "t_lo_all": t_lo,
        "t_hi_all": t_hi,
    }


# revision 72
# speedup vs baseline: 1.3121x; 1.2312x over previous
"""GRU-D cell on 8 Trainium2 NeuronCores (Bass/Tile SPMD kernel).

Data-parallel: the batch dim (16384) is sharded 8 x 2048 across cores;
the six 512x512 weight matrices are replicated. Per core, the 2048-row
shard is processed as 16 tiles of 128 rows (batch on partitions):

  gamma   = exp(-relu(gamma_decay) * delta_t)          (ACT, fused scale)
  x_dec   = x * (m + gx - m*gx)  [+ mu * (1 - q)]      (DVE, bf16)
  h_dec   = gh * h_prev                                (DVE)
  z/r/hh  : PE matmuls, contraction dim on partitions via PE transposes
            of x_dec / h_dec / (r*h_dec); weights replicated in SBUF bf16
  h_new   = h_dec + z * (tanh(hh_pre) - h_dec)         (DVE, fp32 out)

Inputs are cast to bf16 on the host (round-to-nearest) to halve HBM
traffic and let the PE run at full bf16 rate; accumulation stays fp32 in
PSUM. Zero biases / zero mean-imputation / all-ones decay vectors (the
values this problem ships) are detected at runtime and the matching
pre-compiled specialization is used; non-trivial values fall back to a
general variant built from the same emitter.
"""

import os
import threading
from contextlib import ExitStack

import ml_dtypes
import numpy as np

F = 512
P = 128
N_CORES = 8
B = 16384
BC = B // N_CORES          # rows per core
NT = BC // P               # 128-row tiles per core
KC = F // P                # contraction chunks

BF16 = ml_dtypes.bfloat16

_lock = threading.Lock()
_cache = {}

# Tunables (settled via timeline-sim scans + HW loop-delta timing).
_TUNE = {
    "hd_eng": "vector",
    "rh_eng": "vector",
    "xdT_eng": "vector",
    "hdT_eng": "scalar",
    "rhT_eng": "vector",
    "wk_bufs": 5,
    "io_bufs": 3,
    "pt_bufs": 4,
    "pg_bufs": 4,
    "offs": (0, 1, 2, 3, 4),
    "zaff_eng": "gpsimd",
    "d_eng": "vector",
    "zd_eng": "vector",
    "w_split": True,
    "warmup": 24,
    # layout B knobs
    "layout": "A",
    "wkb_bufs": 3,
    "iob_bufs": 3,
    "pgb_bufs": 6,
    "ptb_bufs": 2,
    "offs_b": (0, 1, 2, 3),
    "hnc_eng": "scalar",
}


def _HD_ENG(nc):
    return getattr(nc, _TUNE["hd_eng"])


def _RH_ENG(nc):
    return getattr(nc, _TUNE["rh_eng"])


def _copy(nc, eng, out, in_):
    if eng == "scalar":
        nc.scalar.copy(out, in_)
    else:
        getattr(nc, eng).tensor_copy(out, in_)


def _to_bf16(a: np.ndarray) -> np.ndarray:
    """Fast fp32 -> bf16 with round-to-nearest (ties toward +1 ulp)."""
    a = np.ascontiguousarray(a, dtype=np.float32)
    u = a.view(np.uint32)
    r = (u >> 16) & np.uint32(1)
    out = ((u + np.uint32(0x7FFF) + r) >> 16).astype(np.uint16)
    return out.view(BF16).reshape(a.shape)


def _build_nc(flags, reps=1):
    if _TUNE.get("layout", "A") == "B":
        return _build_nc_b(flags, reps)
    return _build_nc_a(flags, reps)


def _build_nc_a(flags, reps=1):
    """Emit + compile the Bass program for one specialization.

    flags = (shared_gamma, gx_one, gh_one, mu_zero, bz_zero, br_zero, bh_zero)
    reps > 1 unrolls the whole pass multiple times in one NEFF — used only
    for device-time measurement (loop-delta timing amortizes dispatch).
    Returns (nc, in_names_needed).
    """
    import concourse.bass as bass
    import concourse.tile as tile
    from concourse import bacc, mybir
    from concourse.bass import ts
    from concourse.masks import make_identity

    shared_gamma, gx_one, gh_one, mu_zero, bz_zero, br_zero, bh_zero = flags
    BD = mybir.dt.bfloat16
    FD = mybir.dt.float32
    AF = mybir.ActivationFunctionType

    nc = bacc.Bacc("TRN2", target_bir_lowering=False, debug=False,
                   num_devices=N_CORES)

    inp = nc.dram_tensor("inp", [BC, 3 * F], BD, kind="ExternalInput").ap()
    hp = nc.dram_tensor("hp", [BC, F], BD, kind="ExternalInput").ap()
    w = nc.dram_tensor("w", [6, F, F], BD, kind="ExternalInput").ap()
    need = ["inp", "hp", "w"]
    gxr = ghr = mu = bz = br = bh = None
    if not gx_one:
        gxr = nc.dram_tensor("gxr", [F], BD, kind="ExternalInput").ap()
        need.append("gxr")
    if not shared_gamma and not gh_one:
        ghr = nc.dram_tensor("ghr", [F], BD, kind="ExternalInput").ap()
        need.append("ghr")
    if not mu_zero:
        mu = nc.dram_tensor("mu", [F], BD, kind="ExternalInput").ap()
        need.append("mu")
    if not bz_zero:
        bz = nc.dram_tensor("bz", [F], FD, kind="ExternalInput").ap()
        need.append("bz")
    if not br_zero:
        br = nc.dram_tensor("br", [F], FD, kind="ExternalInput").ap()
        need.append("br")
    if not bh_zero:
        bh = nc.dram_tensor("bh", [F], FD, kind="ExternalInput").ap()
        need.append("bh")
    out = nc.dram_tensor("h_new", [BC, F], FD, kind="ExternalOutput").ap()

    inp_r = inp.rearrange("(n p) c -> n p c", p=P)
    hp_r = hp.rearrange("(n p) c -> n p c", p=P)
    out_r = out.rearrange("(n p) c -> n p c", p=P)
    w_r = w.rearrange("w (k p) u -> p w k u", p=P)

    def bcast(pool, dram_ap, dt, name):
        t = pool.tile([P, F], dt, name=name)
        src = bass.AP(tensor=dram_ap.tensor, offset=dram_ap.offset,
                      ap=[[0, P]] + [list(d) for d in dram_ap.ap])
        nc.gpsimd.dma_start(out=t, in_=src)
        return t

    with tile.TileContext(nc) as tc:
        with ExitStack() as ctx:
            consts = ctx.enter_context(tc.tile_pool(name="consts", bufs=1))
            wk = ctx.enter_context(
                tc.tile_pool(name="wk", bufs=_TUNE["wk_bufs"]))
            io_pool = ctx.enter_context(
                tc.tile_pool(name="io", bufs=_TUNE["io_bufs"]))
            pt = ctx.enter_context(
                tc.tile_pool(name="pt", bufs=_TUNE["pt_bufs"], space="PSUM"))
            pg = ctx.enter_context(
                tc.tile_pool(name="pg", bufs=_TUNE["pg_bufs"], space="PSUM"))

            ident = consts.tile([P, P], BD)
            make_identity(nc, ident)
            w_sb = consts.tile([P, 6, KC, F], BD)
            pending_w = {}
            if _TUNE["w_split"]:
                # Weight loads are emitted into the early pipeline rounds
                # (after the first input tiles) so the first transposes and
                # matmuls aren't gated on the full 3MB weight load.
                pending_w = {0: [0, 1], 1: [2, 3], 2: [4, 5]}
            else:
                nc.sync.dma_start(out=w_sb, in_=w_r)
            if _TUNE["warmup"]:
                # The PE clock ramps with sustained use (full speed only
                # after ~3us busy). Run throwaway transposes on the identity
                # during the initial DMA fill so the first real matmuls start
                # at full clock. (Reuses a pt-pool slot; WAW on one tile
                # serializes them on the PE, which is exactly the point.)
                wu_p = pt.tile([P, P], BD, tag="tp", name="wu_p")
                for _ in range(_TUNE["warmup"]):
                    nc.tensor.transpose(wu_p, ident, ident)
            gxb = ghb = mub = bzb = brb = bhb = None
            if gxr is not None:
                gxb = bcast(consts, gxr, BD, "gxb")
            if ghr is not None:
                ghb = bcast(consts, ghr, BD, "ghb")
            if mu is not None:
                mub = bcast(consts, mu, BD, "mub")
            if bz is not None:
                bzb = bcast(consts, bz, FD, "bzb")
            if br is not None:
                # feature-major [128, KC], pre-scaled by 0.5 for the fused
                # tanh(r_pre/2 + b/2) in the feature-major r gate
                brb_fm = consts.tile([P, KC], FD, name="brb_fm")
                nc.gpsimd.dma_start(
                    out=brb_fm, in_=br.rearrange("(c p) -> p c", p=P))
                nc.vector.tensor_scalar(brb_fm, brb_fm, 0.5, None,
                                        mybir.AluOpType.mult)
            if bh is not None:
                bhb = bcast(consts, bh, FD, "bhb")

            if reps > 1:
                loop_ctx = tc.For_i(0, reps, 1)
                loop_ctx.__enter__()

            def stage_a1(i):
                """DMA-in, imputation/decay, xd/hd transposes."""
                it = io_pool.tile([P, 3 * F], BD, tag="it")
                if i == 0:
                    # delta_t slice first: it gates the exp that heads the
                    # whole dependency chain of the first tile.
                    nc.sync.dma_start(out=it[:, 2 * F:], in_=inp_r[i][:, 2 * F:])
                    nc.sync.dma_start(out=it[:, :2 * F], in_=inp_r[i][:, :2 * F])
                else:
                    nc.sync.dma_start(out=it, in_=inp_r[i])
                ht = io_pool.tile([P, F], BD, tag="ht")
                nc.sync.dma_start(out=ht, in_=hp_r[i])
                x_ = it[:, 0:F]
                m_ = it[:, F:2 * F]
                dt_ = it[:, 2 * F:3 * F]

                # gamma_x = exp(-relu(gx) * delta_t)
                g = wk.tile([P, F], BD, tag="g")
                if gx_one:
                    if i == 0:
                        nc.scalar.activation(g[:, :F // 2], dt_[:, :F // 2],
                                             AF.Exp, scale=-1.0)
                        nc.scalar.activation(g[:, F // 2:], dt_[:, F // 2:],
                                             AF.Exp, scale=-1.0)
                    else:
                        nc.scalar.activation(g, dt_, AF.Exp, scale=-1.0)
                else:
                    ga = wk.tile([P, F], BD, tag="ga")
                    nc.vector.tensor_mul(ga, dt_, gxb)
                    nc.scalar.activation(g, ga, AF.Exp, scale=-1.0)
                if shared_gamma:
                    gH = g
                elif gh_one:
                    gH = wk.tile([P, F], BD, tag="gH")
                    nc.scalar.activation(gH, dt_, AF.Exp, scale=-1.0)
                else:
                    gha = wk.tile([P, F], BD, tag="gha")
                    nc.vector.tensor_mul(gha, dt_, ghb)
                    gH = wk.tile([P, F], BD, tag="gH")
                    nc.scalar.activation(gH, gha, AF.Exp, scale=-1.0)

                # x_decayed = x*q (+ mu*(1-q)), q = m + g - m*g
                # For the first tile, run the chain in column halves so the
                # first PE transposes unblock (via subtile deps) ~1us sooner.
                t1 = wk.tile([P, F], BD, tag="t1")
                q = wk.tile([P, F], BD, tag="q")
                q2 = wk.tile([P, F], BD, tag="q2")
                xd = wk.tile([P, F], BD, tag="xd")
                halves = ([slice(0, F // 2), slice(F // 2, F)]
                          if i == 0 else [slice(0, F)])
                for sl in halves:
                    nc.vector.tensor_mul(t1[:, sl], m_[:, sl], g[:, sl])
                    nc.vector.tensor_add(q[:, sl], m_[:, sl], g[:, sl])
                    nc.vector.tensor_sub(q2[:, sl], q[:, sl], t1[:, sl])
                    nc.vector.tensor_mul(xd[:, sl], x_[:, sl], q2[:, sl])
                if not mu_zero:
                    omq = wk.tile([P, F], BD, tag="omq")
                    nc.vector.tensor_scalar(
                        omq, q2, -1.0, 1.0,
                        mybir.AluOpType.mult, mybir.AluOpType.add)
                    muq = wk.tile([P, F], BD, tag="muq")
                    nc.vector.tensor_mul(muq, omq, mub)
                    xd2 = wk.tile([P, F], BD, tag="xd2")
                    nc.vector.tensor_add(xd2, xd, muq)
                    xd = xd2
                hd = wk.tile([P, F], BD, tag="hd")
                _HD_ENG(nc).tensor_mul(hd, gH, ht)
                return dict(xd=xd, hd=hd, i=i)

            def stage_tr(st):
                """PE transposes of xd/hd: [128-batch, 512-f] -> [512-f, 128-b]."""
                xd, hd = st["xd"], st["hd"]
                xdT_p = pt.tile([P, F], BD, tag="tp")
                for k in range(KC):
                    nc.tensor.transpose(xdT_p[:, ts(k, P)], xd[:, ts(k, P)], ident)
                xdT = wk.tile([P, F], BD, tag="xdT")
                _copy(nc, _TUNE["xdT_eng"], xdT, xdT_p)
                hdT_p = pt.tile([P, F], BD, tag="tp")
                for k in range(KC):
                    nc.tensor.transpose(hdT_p[:, ts(k, P)], hd[:, ts(k, P)], ident)
                hdT = wk.tile([P, F], BD, tag="hdT")
                _copy(nc, _TUNE["hdT_eng"], hdT, hdT_p)
                st.update(xdT=xdT, hdT=hdT)
                return st

            def stage_zr(st):
                """z/r gate matmuls, sigmoids, rh."""
                xdT, hdT, hd = st["xdT"], st["hdT"], st["hd"]
                zp = pg.tile([P, F], FD, tag="gp")
                for k in range(KC):
                    nc.tensor.matmul(zp, xdT[:, ts(k, P)], w_sb[:, 0, k, :],
                                     start=(k == 0), stop=False)
                for k in range(KC):
                    nc.tensor.matmul(zp, hdT[:, ts(k, P)], w_sb[:, 1, k, :],
                                     start=False, stop=(k == KC - 1))
                # r gate FEATURE-major: lhsT = weight chunks, rhs = the
                # already-transposed xdT/hdT. The result r^T lands in the
                # exact orientation the hh-gate's U_h matmuls need, so the
                # rh transpose + PSUM->SBUF copy disappear entirely. Same
                # matmul cycle count (32 x N=128 vs 8 x N=512).
                rp = pg.tile([P, F], FD, tag="gp")
                for u in range(KC):
                    for k in range(KC):
                        nc.tensor.matmul(rp[:, ts(u, P)],
                                         w_sb[:, 2, k, ts(u, P)],
                                         xdT[:, ts(k, P)],
                                         start=(k == 0), stop=False)
                    for k in range(KC):
                        nc.tensor.matmul(rp[:, ts(u, P)],
                                         w_sb[:, 3, k, ts(u, P)],
                                         hdT[:, ts(k, P)],
                                         start=False, stop=(k == KC - 1))
                # sigmoid(a) = 0.5 + 0.5*tanh(a/2): keeps every ACT func in
                # the single `exp_and_others` table (Exp/Tanh/Copy) — a
                # Sigmoid would force ~1.3us act-table reloads per switch.
                zt = wk.tile([P, F], BD, tag="zt")
                if bz_zero:
                    nc.scalar.activation(zt, zp, AF.Tanh, scale=0.5)
                else:
                    zb = wk.tile([P, F], FD, tag="zb")
                    nc.vector.tensor_add(zb, zp, bzb)
                    nc.scalar.activation(zt, zb, AF.Tanh, scale=0.5)
                z = wk.tile([P, F], BD, tag="z")
                getattr(nc, _TUNE["zaff_eng"]).tensor_scalar(
                    z, zt, 0.5, 0.5,
                    mybir.AluOpType.mult, mybir.AluOpType.add)
                # rt^T = tanh(r_pre^T/2) — feature-major, bias is now
                # per-partition so the general path fuses it into ACT.
                rt = wk.tile([P, F], BD, tag="rt")
                if br_zero:
                    nc.scalar.activation(rt, rp, AF.Tanh, scale=0.5)
                else:
                    for u in range(KC):
                        nc.scalar.activation(rt[:, ts(u, P)], rp[:, ts(u, P)],
                                             AF.Tanh, scale=0.5,
                                             bias=brb_fm[:, u:u + 1])
                # r = 0.5 + 0.5*rt, and rh = r*hd feeds only (rh @ U_h).
                # The 0.5 factor is folded into U_h host-side, so
                # rh'^T = hd^T + rt^T*hd^T (all feature-major, SBUF only).
                v = wk.tile([P, F], BD, tag="v")
                nc.vector.tensor_mul(v, rt, hdT)
                rhT = wk.tile([P, F], BD, tag="rhT")
                _RH_ENG(nc).tensor_add(rhT, hdT, v)
                st.update(rhT=rhT, z=z)
                return st

            def stage_rt(st):
                """(folded into stage_zr: r is computed feature-major)"""
                return st

            def stage_b(st):
                """hh gate matmuls, tanh, blend, DMA-out.

                The last tile runs in two N=256 column halves so the
                closing tanh->blend->store chain (the kernel's drain tail)
                overlaps the second half's matmuls."""
                xdT, rhT, hd, z, i = (st["xdT"], st["rhT"], st["hd"],
                                      st["z"], st["i"])
                hh = wk.tile([P, F], BD, tag="hh")
                d = wk.tile([P, F], BD, tag="d")
                zd = wk.tile([P, F], BD, tag="zd")
                hn = wk.tile([P, F], FD, tag="hn")
                halves = ([slice(0, F // 2), slice(F // 2, F)]
                          if i == NT - 1 else [slice(0, F)])
                for sl in halves:
                    hpp = pg.tile([P, sl.stop - sl.start], FD, tag="gp",
                                  name="hpp")
                    for k in range(KC):
                        nc.tensor.matmul(hpp, xdT[:, ts(k, P)],
                                         w_sb[:, 4, k, sl],
                                         start=(k == 0), stop=False)
                    for k in range(KC):
                        nc.tensor.matmul(hpp, rhT[:, ts(k, P)],
                                         w_sb[:, 5, k, sl],
                                         start=False, stop=(k == KC - 1))
                    if bh_zero:
                        nc.scalar.activation(hh[:, sl], hpp, AF.Tanh)
                    else:
                        hb = wk.tile([P, F], FD, tag="hb")
                        nc.vector.tensor_add(hb[:, sl], hpp, bhb[:, sl])
                        nc.scalar.activation(hh[:, sl], hb[:, sl], AF.Tanh)

                    # h_new = hd + z*(hh - hd)
                    getattr(nc, _TUNE["d_eng"]).tensor_sub(
                        d[:, sl], hh[:, sl], hd[:, sl])
                    getattr(nc, _TUNE["zd_eng"]).tensor_mul(
                        zd[:, sl], z[:, sl], d[:, sl])
                    nc.vector.tensor_add(hn[:, sl], hd[:, sl], zd[:, sl])
                    nc.sync.dma_start(out=out_r[i][:, sl], in_=hn[:, sl])

            stages = [stage_a1, stage_tr, stage_zr, stage_b]
            offs = _TUNE["offs"]
            sts = {}
            for r in range(NT + max(offs)):
                for s, off in zip(stages, offs):
                    j = r - off
                    if 0 <= j < NT:
                        if s is stage_a1:
                            sts[j] = s(j)
                        else:
                            s(sts[j])
                for j in pending_w.pop(r, ()):
                    nc.sync.dma_start(out=w_sb[:, j], in_=w_r[:, j])
            sts.clear()

            if reps > 1:
                loop_ctx.__exit__(None, None, None)

    nc.compile()
    return nc, need


def _build_nc_b(flags, reps=1):
    """Layout B: feature-major compute via DMA-transposed input loads.

    Inputs land in SBUF already transposed ([feature-part, batch-free],
    512-row batch chunks), so the imputation/decay math, gates and blend
    all run feature-major: no PE input transposes, no PSUM->SBUF copies
    for matmul operands, and biases/decay vectors fuse into ACT as
    per-partition scalars. Only the final h_new needs PE transposes back
    to batch-major. Weights are the stationary matmul operand here.
    """
    import concourse.bass as bass
    import concourse.tile as tile
    from concourse import bacc, mybir
    from concourse.bass import ts
    from concourse.masks import make_identity

    shared_gamma, gx_one, gh_one, mu_zero, bz_zero, br_zero, bh_zero = flags
    BD = mybir.dt.bfloat16
    FD = mybir.dt.float32
    AF = mybir.ActivationFunctionType
    CW = 512               # batch columns per chunk
    NCH = BC // CW         # chunks per core (4)

    nc = bacc.Bacc("TRN2", target_bir_lowering=False, debug=False,
                   num_devices=N_CORES)

    inp = nc.dram_tensor("inp", [BC, 3 * F], BD, kind="ExternalInput").ap()
    hp = nc.dram_tensor("hp", [BC, F], BD, kind="ExternalInput").ap()
    w = nc.dram_tensor("w", [6, F, F], BD, kind="ExternalInput").ap()
    need = ["inp", "hp", "w"]
    gxr = ghr = mu = bz = br = bh = None
    if not gx_one:
        # host passes NEGATED relu'd decay (used as ACT scale)
        gxr = nc.dram_tensor("gxr", [F], FD, kind="ExternalInput").ap()
        need.append("gxr")
    if not shared_gamma and not gh_one:
        ghr = nc.dram_tensor("ghr", [F], FD, kind="ExternalInput").ap()
        need.append("ghr")
    if not mu_zero:
        mu = nc.dram_tensor("mu", [F], FD, kind="ExternalInput").ap()
        need.append("mu")
    if not bz_zero:
        bz = nc.dram_tensor("bz", [F], FD, kind="ExternalInput").ap()
        need.append("bz")
    if not br_zero:
        br = nc.dram_tensor("br", [F], FD, kind="ExternalInput").ap()
        need.append("br")
    if not bh_zero:
        bh = nc.dram_tensor("bh", [F], FD, kind="ExternalInput").ap()
        need.append("bh")
    out = nc.dram_tensor("h_new", [BC, F], FD, kind="ExternalOutput").ap()

    w_r = w.rearrange("w (k p) u -> p w k u", p=P)

    def fmaj(pool, dram_ap, name):
        # [F] vector -> [128, KC] feature-major ([p, c] = v[c*128+p])
        t = pool.tile([P, KC], FD, name=name)
        nc.gpsimd.dma_start(out=t, in_=dram_ap.rearrange("(c p) -> p c", p=P))
        return t

    with tile.TileContext(nc) as tc:
        with ExitStack() as ctx:
            consts = ctx.enter_context(tc.tile_pool(name="consts", bufs=1))
            wk = ctx.enter_context(
                tc.tile_pool(name="wk", bufs=_TUNE["wkb_bufs"]))
            io_pool = ctx.enter_context(
                tc.tile_pool(name="io", bufs=_TUNE["iob_bufs"]))
            pg = ctx.enter_context(
                tc.tile_pool(name="pg", bufs=_TUNE["pgb_bufs"], space="PSUM"))
            pt = ctx.enter_context(
                tc.tile_pool(name="pt", bufs=_TUNE["ptb_bufs"], space="PSUM"))

            ident = consts.tile([P, P], BD)
            make_identity(nc, ident)
            w_sb = consts.tile([P, 6, KC, F], BD)
            pending_w = {0: [0, 1, 2, 3], 1: [4, 5]}

            gxb = ghb = mub = nmub = bzb = brb = bhb = None
            if gxr is not None:
                gxb = fmaj(consts, gxr, "gxb")
            if ghr is not None:
                ghb = fmaj(consts, ghr, "ghb")
            if mu is not None:
                mub = fmaj(consts, mu, "mub")
                nmub = consts.tile([P, KC], FD, name="nmub")
                nc.vector.tensor_scalar(nmub, mub, -1.0, None,
                                        mybir.AluOpType.mult)
            if bz is not None:
                bzb = fmaj(consts, bz, "bzb")   # host pre-scaled by 0.5
            if br is not None:
                brb = fmaj(consts, br, "brb")   # host pre-scaled by 0.5
            if bh is not None:
                bhb = fmaj(consts, bh, "bhb")

            def s_load(c):
                """DMA-transpose x/m/dt/h for batch rows [c*CW, (c+1)*CW)."""
                rows = slice(c * CW, (c + 1) * CW)
                xT = io_pool.tile([P, KC, CW], BD, tag="xT")
                mT = io_pool.tile([P, KC, CW], BD, tag="mT")
                dT = io_pool.tile([P, KC, CW], BD, tag="dT")
                hT = io_pool.tile([P, KC, CW], BD, tag="hT")
                # delta_t first (the exp is the first consumer), then m, x, h
                for f in range(KC):
                    nc.sync.dma_start(
                        out=dT[:, f, :],
                        in_=inp[rows, 2 * F + f * P:2 * F + (f + 1) * P],
                        transpose=True)
                for f in range(KC):
                    nc.sync.dma_start(out=mT[:, f, :],
                                      in_=inp[rows, F + f * P:F + (f + 1) * P],
                                      transpose=True)
                for f in range(KC):
                    nc.sync.dma_start(out=xT[:, f, :],
                                      in_=inp[rows, ts(f, P)], transpose=True)
                for f in range(KC):
                    nc.sync.dma_start(out=hT[:, f, :],
                                      in_=hp[rows, ts(f, P)], transpose=True)
                return dict(xT=xT, mT=mT, dT=dT, hT=hT, c=c)

            def s_imp(st):
                """Imputation + decay, feature-major, mostly in place:
                xT -> x_decayed, hT -> h_decayed, mT/dT scratch."""
                xT, mT, dT, hT = st["xT"], st["mT"], st["dT"], st["hT"]
                g = wk.tile([P, KC, CW], BD, tag="g")
                if gx_one:
                    nc.scalar.activation(g, dT, AF.Exp, scale=-1.0)
                else:
                    for f in range(KC):
                        nc.scalar.activation(g[:, f, :], dT[:, f, :], AF.Exp,
                                             scale=gxb[:, f:f + 1])
                if shared_gamma:
                    gH = g
                elif gh_one:
                    gH = wk.tile([P, KC, CW], BD, tag="gH")
                    nc.scalar.activation(gH, dT, AF.Exp, scale=-1.0)
                else:
                    gH = wk.tile([P, KC, CW], BD, tag="gH")
                    for f in range(KC):
                        nc.scalar.activation(gH[:, f, :], dT[:, f, :], AF.Exp,
                                             scale=ghb[:, f:f + 1])
                # q = m + g - m*g  (dT <- m*g, mT <- q)
                nc.vector.tensor_mul(dT, mT, g)
                nc.vector.tensor_add(mT, mT, g)
                nc.vector.tensor_sub(mT, mT, dT)
                # xd = x*q (+ mu*(1-q))
                nc.vector.tensor_mul(xT, xT, mT)
                if not mu_zero:
                    for f in range(KC):
                        nc.vector.tensor_scalar(
                            dT[:, f, :], mT[:, f, :],
                            nmub[:, f:f + 1], mub[:, f:f + 1],
                            mybir.AluOpType.mult, mybir.AluOpType.add)
                    nc.vector.tensor_add(xT, xT, dT)
                # hd = gH * h
                nc.vector.tensor_mul(hT, gH, hT)
                st["g"] = g
                return st

            def s_zr(st):
                """z and r gates + rh (in place on rt)."""
                xT, hT = st["xT"], st["hT"]
                zt = wk.tile([P, KC, CW], BD, tag="zt")
                for u in range(KC):
                    zp = pg.tile([P, CW], FD, tag="gp")
                    for f in range(KC):
                        nc.tensor.matmul(zp, w_sb[:, 0, f, ts(u, P)],
                                         xT[:, f, :],
                                         start=(f == 0), stop=False)
                    for f in range(KC):
                        nc.tensor.matmul(zp, w_sb[:, 1, f, ts(u, P)],
                                         hT[:, f, :],
                                         start=False, stop=(f == KC - 1))
                    if bz_zero:
                        nc.scalar.activation(zt[:, u, :], zp, AF.Tanh,
                                             scale=0.5)
                    else:
                        nc.scalar.activation(zt[:, u, :], zp, AF.Tanh,
                                             scale=0.5, bias=bzb[:, u:u + 1])
                z = wk.tile([P, KC, CW], BD, tag="z")
                getattr(nc, _TUNE["zaff_eng"]).tensor_scalar(
                    z, zt, 0.5, 0.5,
                    mybir.AluOpType.mult, mybir.AluOpType.add)
                rt = wk.tile([P, KC, CW], BD, tag="rt")
                for u in range(KC):
                    rp = pg.tile([P, CW], FD, tag="gp")
                    for f in range(KC):
                        nc.tensor.matmul(rp, w_sb[:, 2, f, ts(u, P)],
                                         xT[:, f, :],
                                         start=(f == 0), stop=False)
                    for f in range(KC):
                        nc.tensor.matmul(rp, w_sb[:, 3, f, ts(u, P)],
                                         hT[:, f, :],
                                         start=False, stop=(f == KC - 1))
                    if br_zero:
                        nc.scalar.activation(rt[:, u, :], rp, AF.Tanh,
                                             scale=0.5)
                    else:
                        nc.scalar.activation(rt[:, u, :], rp, AF.Tanh,
                                             scale=0.5, bias=brb[:, u:u + 1])
                # rh' = hd + rt*hd  (U_h host-scaled by 0.5); in place on rt
                nc.vector.tensor_mul(rt, rt, hT)
                nc.vector.tensor_add(rt, rt, hT)
                st.update(z=z, rh=rt)
                return st

            def s_hh(st):
                """hh gate, blend (in place -> h_new^T), transpose, store."""
                xT, hT, rh, z, c = st["xT"], st["hT"], st["rh"], st["z"], st["c"]
                hhT = wk.tile([P, KC, CW], BD, tag="hhT")
                for u in range(KC):
                    hpp = pg.tile([P, CW], FD, tag="gp")
                    for f in range(KC):
                        nc.tensor.matmul(hpp, w_sb[:, 4, f, ts(u, P)],
                                         xT[:, f, :],
                                         start=(f == 0), stop=False)
                    for f in range(KC):
                        nc.tensor.matmul(hpp, w_sb[:, 5, f, ts(u, P)],
                                         rh[:, f, :],
                                         start=False, stop=(f == KC - 1))
                    if bh_zero:
                        nc.scalar.activation(hhT[:, u, :], hpp, AF.Tanh)
                    else:
                        nc.scalar.activation(hhT[:, u, :], hpp, AF.Tanh,
                                             bias=bhb[:, u:u + 1])
                # h_new = hd + z*(hh - hd), in place on hhT
                nc.vector.tensor_sub(hhT, hhT, hT)
                nc.vector.tensor_mul(hhT, z, hhT)
                nc.vector.tensor_add(hhT, hT, hhT)
                # transpose back to batch-major and store
                for bs in range(CW // P):
                    hn_p = pt.tile([P, F], BD, tag="tp")
                    for u in range(KC):
                        nc.tensor.transpose(hn_p[:, ts(u, P)],
                                            hhT[:, u, ts(bs, P)], ident)
                    hn = wk.tile([P, F], FD, tag="hn")
                    _copy(nc, _TUNE["hnc_eng"], hn, hn_p)
                    nc.sync.dma_start(
                        out=out[c * CW + bs * P:c * CW + (bs + 1) * P, :],
                        in_=hn)

            if reps > 1:
                loop_ctx = tc.For_i(0, reps, 1)
                loop_ctx.__enter__()

            stages = [s_load, s_imp, s_zr, s_hh]
            offs = _TUNE["offs_b"]
            sts = {}
            for r in range(NCH + max(offs)):
                if r > 0:
                    for j in pending_w.pop(r - 1, ()):
                        nc.sync.dma_start(out=w_sb[:, j], in_=w_r[:, j])
                for s, off in zip(stages, offs):
                    j = r - off
                    if 0 <= j < NCH:
                        if s is s_load:
                            sts[j] = s(j)
                        else:
                            s(sts[j])
            sts.clear()

            if reps > 1:
                loop_ctx.__exit__(None, None, None)

    nc.compile()
    return nc, need


class _Runner:
    """Cached jitted shard_map executor (mirrors bass2jax.run_bass_via_pjrt).

    Inputs are passed as FULL concatenated arrays (shape[0] = 8x the
    per-core shard), which shard_map slices along axis 0 — a per-core
    shard list concatenated on axis 0 is just the original full tensor.
    """

    def __init__(self, nc):
        import jax
        from concourse import bass2jax, mybir

        bass2jax.install_neuronx_cc_hook()

        partition_name = (nc.partition_id_tensor.name
                          if nc.partition_id_tensor else None)
        in_names, out_names, out_avals = [], [], []
        for alloc in nc.m.functions[0].allocations:
            if not isinstance(alloc, mybir.MemoryLocationSet):
                continue
            name = alloc.memorylocations[0].name
            if alloc.kind == "ExternalInput":
                if name != partition_name:
                    in_names.append(name)
            elif alloc.kind == "ExternalOutput":
                out_names.append(name)
                out_avals.append(jax.core.ShapedArray(
                    tuple(alloc.tensor_shape), mybir.dt.np(alloc.dtype)))
        n_params = len(in_names)
        n_outs = len(out_names)
        all_names = tuple(in_names) + tuple(out_names)
        if partition_name is not None:
            all_names = all_names + (partition_name,)

        def _body(*args):
            operands = list(args)
            if partition_name is not None:
                operands.append(bass2jax.partition_id_tensor())
            outs = bass2jax._bass_exec_p.bind(
                *operands,
                out_avals=tuple(out_avals),
                in_names=all_names,
                out_names=tuple(out_names),
                lowering_input_output_aliases=(),
                sim_require_finite=True,
                sim_require_nnan=True,
                nc=nc,
            )
            return tuple(outs)

        devices = jax.devices()[:N_CORES]
        self.mesh = bass2jax.Mesh(np.asarray(devices), ("core",))
        self.spec = bass2jax.PartitionSpec("core")
        rep = bass2jax.PartitionSpec()
        # inp/hp are batch-sharded; everything else is replicated.
        self.in_specs = tuple(
            self.spec if name in ("inp", "hp") else rep for name in in_names)
        self.sharded = jax.jit(
            bass2jax.shard_map(
                _body, mesh=self.mesh,
                in_specs=self.in_specs + (self.spec,) * n_outs,
                out_specs=(self.spec,) * n_outs,
                check_rep=False),
            donate_argnums=tuple(range(n_params, n_params + n_outs)),
            keep_unused=True,
        )
        self.in_names = in_names
        self.out_names = out_names
        self.out_avals = out_avals

    def zeros(self):
        return [np.zeros((N_CORES * a.shape[0], *a.shape[1:]), a.dtype)
                for a in self.out_avals]

    def __call__(self, full_arrays):
        """full_arrays: dict name -> full (8x-shard-concat) np array."""
        concat_in = [full_arrays[name] for name in self.in_names]
        out_arrs = self.sharded(*concat_in, *self.zeros())
        return np.asarray(out_arrs[0])


def _get_runner(flags, reps=1):
    with _lock:
        key = (flags, reps)
        if key not in _cache:
            nc, need = _build_nc(flags, reps)
            _cache[key] = (_Runner(nc), need)
        return _cache[key]


def _host_arrays(inputs):
    """Host-side value prep shared by kernel() and the test harnesses.
    Returns (flags, full_arrays dict keyed by dram-tensor name) — extras
    included unconditionally; callers filter by the program's `need`."""
    inp = np.ascontiguousarray(inputs["inputs"], dtype=np.float32)
    hp = np.ascontiguousarray(inputs["h_prev"], dtype=np.float32)
    W = np.stack([np.asarray(inputs[k], dtype=np.float32)
                  for k in ("W_z", "U_z", "W_r", "U_r", "W_h", "U_h")])
    # The r-gate is computed as rt = tanh(r_pre/2) with rh' = hd + rt*hd
    # = 2*(r*hd); the missing 0.5 of sigmoid(a) = 0.5 + 0.5*tanh(a/2) is
    # folded into U_h here (rh' only ever feeds rh' @ U_h).
    W[5] *= 0.5
    gxr = np.maximum(np.asarray(inputs["gamma_x_decay"], np.float32), 0.0)
    ghr = np.maximum(np.asarray(inputs["gamma_h_decay"], np.float32), 0.0)
    mu = np.asarray(inputs["mean_imputation"], np.float32)
    bz = np.asarray(inputs["b_z"], np.float32)
    br = np.asarray(inputs["b_r"], np.float32)
    bh = np.asarray(inputs["b_h"], np.float32)

    gx_one = bool(np.all(gxr == 1.0))
    gh_one = bool(np.all(ghr == 1.0))
    shared = bool(np.array_equal(gxr, ghr))
    flags = (shared, gx_one, gh_one,
             not mu.any(), not bz.any(), not br.any(), not bh.any())

    full = {"inp": _to_bf16(inp), "hp": _to_bf16(hp), "w": _to_bf16(W)}
    if _TUNE.get("layout", "A") == "B":
        # B contract: decays are negated fp32 ACT scales; z/r biases are
        # pre-scaled by 0.5 (fused into tanh(a/2 + b/2)).
        full["gxr"] = -gxr
        full["ghr"] = -ghr
        full["mu"] = mu
        full["bz"] = 0.5 * bz
        full["br"] = 0.5 * br
        full["bh"] = bh
    else:
        full["gxr"] = _to_bf16(gxr)
        full["ghr"] = _to_bf16(ghr)
        full["mu"] = _to_bf16(mu)
        full["bz"] = bz
        full["br"] = br
        full["bh"] = bh
    return flags, full


def _prep(inputs):
    """Returns (runner, full_arrays dict keyed by dram-tensor name)."""
    flags, full = _host_arrays(inputs)
    run, need = _get_runner(flags)
    return run, {k: v for k, v in full.items()
                 if k in ("inp", "hp", "w") or k in need}


def kernel(**inputs) -> np.ndarray:
    run, full = _prep(inputs)
    return run(full)


def _timed_calls(run, full, iters):
    import time

    import jax
    from jax.sharding import NamedSharding

    out_sh = NamedSharding(run.mesh, run.spec)
    dev_in = [
        jax.device_put(full[name], NamedSharding(run.mesh, spec))
        for name, spec in zip(run.in_names, run.in_specs)
    ]
    zero_sets = [[jax.device_put(z, out_sh) for z in run.zeros()]
                 for _ in range(iters)]
    jax.block_until_ready(dev_in)
    jax.block_until_ready(zero_sets)
    jax.block_until_ready(run.sharded(*dev_in, *run.zeros()))  # warm-up
    times = []
    for i in range(iters):
        t0 = time.perf_counter_ns()
        out = run.sharded(*dev_in, *zero_sets[i])
        jax.block_until_ready(out)
        times.append(time.perf_counter_ns() - t0)
    times.sort()
    return times


def bench_device(inputs, iters: int = 14, reps_lo: int = 8, reps_hi: int = 64):
    """Estimate on-device per-pass execution time via loop-delta timing:
    two looped variants of the kernel (hardware For-loop in one NEFF, same
    code shape) amortize the large axon dispatch overhead; per-pass time =
    (median(T_hi) - median(T_lo)) / (reps_hi - reps_lo). Medians because
    the dispatch noise has heavy two-sided outliers. The loop back-edge is
    a full barrier, so this measures a complete pass including pipeline
    fill/drain — a slight over-estimate of the streamed single-shot time."""
    run1, full = _prep(inputs)
    flags_key = next(k for k in _cache if _cache[k][0] is run1)[0]
    run_lo, _ = _get_runner(flags_key, reps=reps_lo)
    run_hi, _ = _get_runner(flags_key, reps=reps_hi)

    t_lo = _timed_calls(run_lo, full, iters)
    t_hi = _timed_calls(run_hi, full, iters)
    med_lo = t_lo[len(t_lo) // 2]
    med_hi = t_hi[len(t_hi) // 2]
    per_pass = (med_hi - med_lo) / (reps_hi - reps_lo)
    return {
        "per_pass_ns": int(per_pass),
        "t_lo_med_ns": med_lo,
        "t_hi_med_ns": med_hi,
        "t_lo_all": t_lo,
        "t_hi_all": t_hi,
    }


# revision 76
# speedup vs baseline: 1.3469x; 1.0265x over previous
"""GRU-D cell on 8 Trainium2 NeuronCores (Bass/Tile SPMD kernel).

Data-parallel: the batch dim (16384) is sharded 8 x 2048 across cores;
the six 512x512 weight matrices are replicated. Per core, the 2048-row
shard is processed as 16 tiles of 128 rows (batch on partitions):

  gamma   = exp(-relu(gamma_decay) * delta_t)          (ACT, fused scale)
  x_dec   = x * (m + gx - m*gx)  [+ mu * (1 - q)]      (DVE, bf16)
  h_dec   = gh * h_prev                                (DVE)
  z/r/hh  : PE matmuls, contraction dim on partitions via PE transposes
            of x_dec / h_dec / (r*h_dec); weights replicated in SBUF bf16
  h_new   = h_dec + z * (tanh(hh_pre) - h_dec)         (DVE, fp32 out)

Inputs are cast to bf16 on the host (round-to-nearest) to halve HBM
traffic and let the PE run at full bf16 rate; accumulation stays fp32 in
PSUM. Zero biases / zero mean-imputation / all-ones decay vectors (the
values this problem ships) are detected at runtime and the matching
pre-compiled specialization is used; non-trivial values fall back to a
general variant built from the same emitter.
"""

import os
import threading
from contextlib import ExitStack

import ml_dtypes
import numpy as np

F = 512
P = 128
N_CORES = 8
B = 16384
BC = B // N_CORES          # rows per core
NT = BC // P               # 128-row tiles per core
KC = F // P                # contraction chunks

BF16 = ml_dtypes.bfloat16

_lock = threading.Lock()
_cache = {}

# Tunables (settled via timeline-sim scans + HW loop-delta timing).
_TUNE = {
    "hd_eng": "vector",
    "rh_eng": "vector",
    "xdT_eng": "vector",
    "hdT_eng": "scalar",
    "rhT_eng": "vector",
    "wk_bufs": 5,
    "io_bufs": 3,
    "pt_bufs": 4,
    "pg_bufs": 4,
    "offs": (0, 1, 2, 3, 4),
    "zaff_eng": "gpsimd",
    "d_eng": "vector",
    "zd_eng": "vector",
    "w_split": True,
    "warmup": 24,
    "fill_rounds": 4,
    "fill_warmup": 0,  # neutral in the cost model; keep the verified stream
    # layout B knobs
    "layout": "A",
    "wkb_bufs": 3,
    "iob_bufs": 3,
    "pgb_bufs": 6,
    "ptb_bufs": 2,
    "offs_b": (0, 1, 2, 3),
    "hnc_eng": "scalar",
}


def _HD_ENG(nc):
    return getattr(nc, _TUNE["hd_eng"])


def _RH_ENG(nc):
    return getattr(nc, _TUNE["rh_eng"])


def _copy(nc, eng, out, in_):
    if eng == "scalar":
        nc.scalar.copy(out, in_)
    else:
        getattr(nc, eng).tensor_copy(out, in_)


def _to_bf16(a: np.ndarray) -> np.ndarray:
    """Fast fp32 -> bf16 with round-to-nearest (ties toward +1 ulp)."""
    a = np.ascontiguousarray(a, dtype=np.float32)
    u = a.view(np.uint32)
    r = (u >> 16) & np.uint32(1)
    out = ((u + np.uint32(0x7FFF) + r) >> 16).astype(np.uint16)
    return out.view(BF16).reshape(a.shape)


def _build_nc(flags, reps=1):
    if _TUNE.get("layout", "A") == "B":
        return _build_nc_b(flags, reps)
    return _build_nc_a(flags, reps)


def _build_nc_a(flags, reps=1):
    """Emit + compile the Bass program for one specialization.

    flags = (shared_gamma, gx_one, gh_one, mu_zero, bz_zero, br_zero, bh_zero)
    reps > 1 unrolls the whole pass multiple times in one NEFF — used only
    for device-time measurement (loop-delta timing amortizes dispatch).
    Returns (nc, in_names_needed).
    """
    import concourse.bass as bass
    import concourse.tile as tile
    from concourse import bacc, mybir
    from concourse.bass import ts
    from concourse.masks import make_identity

    shared_gamma, gx_one, gh_one, mu_zero, bz_zero, br_zero, bh_zero = flags
    BD = mybir.dt.bfloat16
    FD = mybir.dt.float32
    AF = mybir.ActivationFunctionType

    nc = bacc.Bacc("TRN2", target_bir_lowering=False, debug=False,
                   num_devices=N_CORES)

    inp = nc.dram_tensor("inp", [BC, 3 * F], BD, kind="ExternalInput").ap()
    hp = nc.dram_tensor("hp", [BC, F], BD, kind="ExternalInput").ap()
    w = nc.dram_tensor("w", [6, F, F], BD, kind="ExternalInput").ap()
    need = ["inp", "hp", "w"]
    gxr = ghr = mu = bz = br = bh = None
    if not gx_one:
        gxr = nc.dram_tensor("gxr", [F], BD, kind="ExternalInput").ap()
        need.append("gxr")
    if not shared_gamma and not gh_one:
        ghr = nc.dram_tensor("ghr", [F], BD, kind="ExternalInput").ap()
        need.append("ghr")
    if not mu_zero:
        mu = nc.dram_tensor("mu", [F], BD, kind="ExternalInput").ap()
        need.append("mu")
    if not bz_zero:
        bz = nc.dram_tensor("bz", [F], FD, kind="ExternalInput").ap()
        need.append("bz")
    if not br_zero:
        br = nc.dram_tensor("br", [F], FD, kind="ExternalInput").ap()
        need.append("br")
    if not bh_zero:
        bh = nc.dram_tensor("bh", [F], FD, kind="ExternalInput").ap()
        need.append("bh")
    out = nc.dram_tensor("h_new", [BC, F], FD, kind="ExternalOutput").ap()

    inp_r = inp.rearrange("(n p) c -> n p c", p=P)
    hp_r = hp.rearrange("(n p) c -> n p c", p=P)
    out_r = out.rearrange("(n p) c -> n p c", p=P)
    w_r = w.rearrange("w (k p) u -> p w k u", p=P)

    def bcast(pool, dram_ap, dt, name):
        t = pool.tile([P, F], dt, name=name)
        src = bass.AP(tensor=dram_ap.tensor, offset=dram_ap.offset,
                      ap=[[0, P]] + [list(d) for d in dram_ap.ap])
        nc.gpsimd.dma_start(out=t, in_=src)
        return t

    with tile.TileContext(nc) as tc:
        with ExitStack() as ctx:
            consts = ctx.enter_context(tc.tile_pool(name="consts", bufs=1))
            wk = ctx.enter_context(
                tc.tile_pool(name="wk", bufs=_TUNE["wk_bufs"]))
            io_pool = ctx.enter_context(
                tc.tile_pool(name="io", bufs=_TUNE["io_bufs"]))
            pt = ctx.enter_context(
                tc.tile_pool(name="pt", bufs=_TUNE["pt_bufs"], space="PSUM"))
            pg = ctx.enter_context(
                tc.tile_pool(name="pg", bufs=_TUNE["pg_bufs"], space="PSUM"))

            ident = consts.tile([P, P], BD)
            make_identity(nc, ident)
            w_sb = consts.tile([P, 6, KC, F], BD)
            pending_w = {}
            if _TUNE["w_split"]:
                # Weight loads are emitted into the early pipeline rounds
                # (after the first input tiles) so the first transposes and
                # matmuls aren't gated on the full 3MB weight load.
                pending_w = {0: [0, 1], 1: [2, 3], 2: [4, 5]}
            else:
                nc.sync.dma_start(out=w_sb, in_=w_r)
            if _TUNE["warmup"] or _TUNE["fill_warmup"]:
                # The PE clock ramps with sustained use (full speed only
                # after ~3us busy). Run throwaway transposes on the identity
                # during the initial DMA fill so the first real matmuls start
                # at full clock. (Reuses a pt-pool slot; WAW on one tile
                # serializes them on the PE, which is exactly the point.)
                wu_p = pt.tile([P, P], BD, tag="tp", name="wu_p")
                for _ in range(_TUNE["warmup"]):
                    nc.tensor.transpose(wu_p, ident, ident)
            gxb = ghb = mub = bzb = brb = bhb = None
            if gxr is not None:
                gxb = bcast(consts, gxr, BD, "gxb")
            if ghr is not None:
                ghb = bcast(consts, ghr, BD, "ghb")
            if mu is not None:
                mub = bcast(consts, mu, BD, "mub")
            if bz is not None:
                bzb = bcast(consts, bz, FD, "bzb")
            if br is not None:
                # feature-major [128, KC], pre-scaled by 0.5 for the fused
                # tanh(r_pre/2 + b/2) in the feature-major r gate
                brb_fm = consts.tile([P, KC], FD, name="brb_fm")
                nc.gpsimd.dma_start(
                    out=brb_fm, in_=br.rearrange("(c p) -> p c", p=P))
                nc.vector.tensor_scalar(brb_fm, brb_fm, 0.5, None,
                                        mybir.AluOpType.mult)
            if bh is not None:
                bhb = bcast(consts, bh, FD, "bhb")

            if reps > 1:
                loop_ctx = tc.For_i(0, reps, 1)
                loop_ctx.__enter__()

            def stage_a1(i):
                """DMA-in, imputation/decay, xd/hd transposes."""
                it = io_pool.tile([P, 3 * F], BD, tag="it")
                if i == 0:
                    # delta_t slice first: it gates the exp that heads the
                    # whole dependency chain of the first tile.
                    nc.sync.dma_start(out=it[:, 2 * F:], in_=inp_r[i][:, 2 * F:])
                    nc.sync.dma_start(out=it[:, :2 * F], in_=inp_r[i][:, :2 * F])
                else:
                    nc.sync.dma_start(out=it, in_=inp_r[i])
                ht = io_pool.tile([P, F], BD, tag="ht")
                nc.sync.dma_start(out=ht, in_=hp_r[i])
                x_ = it[:, 0:F]
                m_ = it[:, F:2 * F]
                dt_ = it[:, 2 * F:3 * F]

                # gamma_x = exp(-relu(gx) * delta_t)
                g = wk.tile([P, F], BD, tag="g")
                if gx_one:
                    if i == 0:
                        nc.scalar.activation(g[:, :F // 2], dt_[:, :F // 2],
                                             AF.Exp, scale=-1.0)
                        nc.scalar.activation(g[:, F // 2:], dt_[:, F // 2:],
                                             AF.Exp, scale=-1.0)
                    else:
                        nc.scalar.activation(g, dt_, AF.Exp, scale=-1.0)
                else:
                    ga = wk.tile([P, F], BD, tag="ga")
                    nc.vector.tensor_mul(ga, dt_, gxb)
                    nc.scalar.activation(g, ga, AF.Exp, scale=-1.0)
                if shared_gamma:
                    gH = g
                elif gh_one:
                    gH = wk.tile([P, F], BD, tag="gH")
                    nc.scalar.activation(gH, dt_, AF.Exp, scale=-1.0)
                else:
                    gha = wk.tile([P, F], BD, tag="gha")
                    nc.vector.tensor_mul(gha, dt_, ghb)
                    gH = wk.tile([P, F], BD, tag="gH")
                    nc.scalar.activation(gH, gha, AF.Exp, scale=-1.0)

                # x_decayed = x*q (+ mu*(1-q)), q = m + g - m*g
                # For the first tile, run the chain in column halves so the
                # first PE transposes unblock (via subtile deps) ~1us sooner.
                t1 = wk.tile([P, F], BD, tag="t1")
                q = wk.tile([P, F], BD, tag="q")
                q2 = wk.tile([P, F], BD, tag="q2")
                xd = wk.tile([P, F], BD, tag="xd")
                halves = ([slice(0, F // 2), slice(F // 2, F)]
                          if i == 0 else [slice(0, F)])
                for sl in halves:
                    nc.vector.tensor_mul(t1[:, sl], m_[:, sl], g[:, sl])
                    nc.vector.tensor_add(q[:, sl], m_[:, sl], g[:, sl])
                    nc.vector.tensor_sub(q2[:, sl], q[:, sl], t1[:, sl])
                    nc.vector.tensor_mul(xd[:, sl], x_[:, sl], q2[:, sl])
                if not mu_zero:
                    omq = wk.tile([P, F], BD, tag="omq")
                    nc.vector.tensor_scalar(
                        omq, q2, -1.0, 1.0,
                        mybir.AluOpType.mult, mybir.AluOpType.add)
                    muq = wk.tile([P, F], BD, tag="muq")
                    nc.vector.tensor_mul(muq, omq, mub)
                    xd2 = wk.tile([P, F], BD, tag="xd2")
                    nc.vector.tensor_add(xd2, xd, muq)
                    xd = xd2
                hd = wk.tile([P, F], BD, tag="hd")
                _HD_ENG(nc).tensor_mul(hd, gH, ht)
                return dict(xd=xd, hd=hd, i=i)

            def stage_tr(st):
                """PE transposes of xd/hd: [128-batch, 512-f] -> [512-f, 128-b]."""
                xd, hd = st["xd"], st["hd"]
                xdT_p = pt.tile([P, F], BD, tag="tp")
                for k in range(KC):
                    nc.tensor.transpose(xdT_p[:, ts(k, P)], xd[:, ts(k, P)], ident)
                xdT = wk.tile([P, F], BD, tag="xdT")
                _copy(nc, _TUNE["xdT_eng"], xdT, xdT_p)
                hdT_p = pt.tile([P, F], BD, tag="tp")
                for k in range(KC):
                    nc.tensor.transpose(hdT_p[:, ts(k, P)], hd[:, ts(k, P)], ident)
                hdT = wk.tile([P, F], BD, tag="hdT")
                _copy(nc, _TUNE["hdT_eng"], hdT, hdT_p)
                st.update(xdT=xdT, hdT=hdT)
                return st

            def stage_zr(st):
                """z/r gate matmuls, sigmoids, rh."""
                xdT, hdT, hd = st["xdT"], st["hdT"], st["hd"]
                zp = pg.tile([P, F], FD, tag="gp")
                for k in range(KC):
                    nc.tensor.matmul(zp, xdT[:, ts(k, P)], w_sb[:, 0, k, :],
                                     start=(k == 0), stop=False)
                for k in range(KC):
                    nc.tensor.matmul(zp, hdT[:, ts(k, P)], w_sb[:, 1, k, :],
                                     start=False, stop=(k == KC - 1))
                # r gate FEATURE-major: lhsT = weight chunks, rhs = the
                # already-transposed xdT/hdT. The result r^T lands in the
                # exact orientation the hh-gate's U_h matmuls need, so the
                # rh transpose + PSUM->SBUF copy disappear entirely. Same
                # matmul cycle count (32 x N=128 vs 8 x N=512).
                rp = pg.tile([P, F], FD, tag="gp")
                for u in range(KC):
                    for k in range(KC):
                        nc.tensor.matmul(rp[:, ts(u, P)],
                                         w_sb[:, 2, k, ts(u, P)],
                                         xdT[:, ts(k, P)],
                                         start=(k == 0), stop=False)
                    for k in range(KC):
                        nc.tensor.matmul(rp[:, ts(u, P)],
                                         w_sb[:, 3, k, ts(u, P)],
                                         hdT[:, ts(k, P)],
                                         start=False, stop=(k == KC - 1))
                # sigmoid(a) = 0.5 + 0.5*tanh(a/2): keeps every ACT func in
                # the single `exp_and_others` table (Exp/Tanh/Copy) — a
                # Sigmoid would force ~1.3us act-table reloads per switch.
                zt = wk.tile([P, F], BD, tag="zt")
                if bz_zero:
                    nc.scalar.activation(zt, zp, AF.Tanh, scale=0.5)
                else:
                    zb = wk.tile([P, F], FD, tag="zb")
                    nc.vector.tensor_add(zb, zp, bzb)
                    nc.scalar.activation(zt, zb, AF.Tanh, scale=0.5)
                z = wk.tile([P, F], BD, tag="z")
                getattr(nc, _TUNE["zaff_eng"]).tensor_scalar(
                    z, zt, 0.5, 0.5,
                    mybir.AluOpType.mult, mybir.AluOpType.add)
                # rt^T = tanh(r_pre^T/2) — feature-major, bias is now
                # per-partition so the general path fuses it into ACT.
                rt = wk.tile([P, F], BD, tag="rt")
                if br_zero:
                    nc.scalar.activation(rt, rp, AF.Tanh, scale=0.5)
                else:
                    for u in range(KC):
                        nc.scalar.activation(rt[:, ts(u, P)], rp[:, ts(u, P)],
                                             AF.Tanh, scale=0.5,
                                             bias=brb_fm[:, u:u + 1])
                # r = 0.5 + 0.5*rt, and rh = r*hd feeds only (rh @ U_h).
                # The 0.5 factor is folded into U_h host-side, so
                # rh'^T = hd^T + rt^T*hd^T (all feature-major, SBUF only).
                v = wk.tile([P, F], BD, tag="v")
                nc.vector.tensor_mul(v, rt, hdT)
                rhT = wk.tile([P, F], BD, tag="rhT")
                _RH_ENG(nc).tensor_add(rhT, hdT, v)
                st.update(rhT=rhT, z=z)
                return st

            def stage_rt(st):
                """(folded into stage_zr: r is computed feature-major)"""
                return st

            def stage_b(st):
                """hh gate matmuls, tanh, blend, DMA-out.

                The last tile runs in two N=256 column halves so the
                closing tanh->blend->store chain (the kernel's drain tail)
                overlaps the second half's matmuls."""
                xdT, rhT, hd, z, i = (st["xdT"], st["rhT"], st["hd"],
                                      st["z"], st["i"])
                hh = wk.tile([P, F], BD, tag="hh")
                d = wk.tile([P, F], BD, tag="d")
                zd = wk.tile([P, F], BD, tag="zd")
                hn = wk.tile([P, F], FD, tag="hn")
                halves = ([slice(0, F // 2), slice(F // 2, F)]
                          if i == NT - 1 else [slice(0, F)])
                for sl in halves:
                    hpp = pg.tile([P, sl.stop - sl.start], FD, tag="gp",
                                  name="hpp")
                    for k in range(KC):
                        nc.tensor.matmul(hpp, xdT[:, ts(k, P)],
                                         w_sb[:, 4, k, sl],
                                         start=(k == 0), stop=False)
                    for k in range(KC):
                        nc.tensor.matmul(hpp, rhT[:, ts(k, P)],
                                         w_sb[:, 5, k, sl],
                                         start=False, stop=(k == KC - 1))
                    if bh_zero:
                        nc.scalar.activation(hh[:, sl], hpp, AF.Tanh)
                    else:
                        hb = wk.tile([P, F], FD, tag="hb")
                        nc.vector.tensor_add(hb[:, sl], hpp, bhb[:, sl])
                        nc.scalar.activation(hh[:, sl], hb[:, sl], AF.Tanh)

                    # h_new = hd + z*(hh - hd)
                    getattr(nc, _TUNE["d_eng"]).tensor_sub(
                        d[:, sl], hh[:, sl], hd[:, sl])
                    getattr(nc, _TUNE["zd_eng"]).tensor_mul(
                        zd[:, sl], z[:, sl], d[:, sl])
                    nc.vector.tensor_add(hn[:, sl], hd[:, sl], zd[:, sl])
                    nc.sync.dma_start(out=out_r[i][:, sl], in_=hn[:, sl])

            stages = [stage_a1, stage_tr, stage_zr, stage_b]
            offs = _TUNE["offs"]
            sts = {}
            for r in range(NT + max(offs)):
                for s, off in zip(stages, offs):
                    j = r - off
                    if 0 <= j < NT:
                        if s is stage_a1:
                            sts[j] = s(j)
                        else:
                            s(sts[j])
                for j in pending_w.pop(r, ()):
                    nc.sync.dma_start(out=w_sb[:, j], in_=w_r[:, j])
                if r < _TUNE["fill_rounds"]:
                    # Keep the PE clocked and busy through the pipeline-fill
                    # region: idle gaps reset the p-state ramp, making the
                    # next ~3us of matmuls run at half clock.
                    for _ in range(_TUNE["fill_warmup"]):
                        nc.tensor.transpose(wu_p, ident, ident)
            sts.clear()

            if reps > 1:
                loop_ctx.__exit__(None, None, None)

    nc.compile()
    return nc, need


def _build_nc_b(flags, reps=1):
    """Layout B: feature-major compute via DMA-transposed input loads.

    Inputs land in SBUF already transposed ([feature-part, batch-free],
    512-row batch chunks), so the imputation/decay math, gates and blend
    all run feature-major: no PE input transposes, no PSUM->SBUF copies
    for matmul operands, and biases/decay vectors fuse into ACT as
    per-partition scalars. Only the final h_new needs PE transposes back
    to batch-major. Weights are the stationary matmul operand here.
    """
    import concourse.bass as bass
    import concourse.tile as tile
    from concourse import bacc, mybir
    from concourse.bass import ts
    from concourse.masks import make_identity

    shared_gamma, gx_one, gh_one, mu_zero, bz_zero, br_zero, bh_zero = flags
    BD = mybir.dt.bfloat16
    FD = mybir.dt.float32
    AF = mybir.ActivationFunctionType
    CW = 512               # batch columns per chunk
    NCH = BC // CW         # chunks per core (4)

    nc = bacc.Bacc("TRN2", target_bir_lowering=False, debug=False,
                   num_devices=N_CORES)

    inp = nc.dram_tensor("inp", [BC, 3 * F], BD, kind="ExternalInput").ap()
    hp = nc.dram_tensor("hp", [BC, F], BD, kind="ExternalInput").ap()
    w = nc.dram_tensor("w", [6, F, F], BD, kind="ExternalInput").ap()
    need = ["inp", "hp", "w"]
    gxr = ghr = mu = bz = br = bh = None
    if not gx_one:
        # host passes NEGATED relu'd decay (used as ACT scale)
        gxr = nc.dram_tensor("gxr", [F], FD, kind="ExternalInput").ap()
        need.append("gxr")
    if not shared_gamma and not gh_one:
        ghr = nc.dram_tensor("ghr", [F], FD, kind="ExternalInput").ap()
        need.append("ghr")
    if not mu_zero:
        mu = nc.dram_tensor("mu", [F], FD, kind="ExternalInput").ap()
        need.append("mu")
    if not bz_zero:
        bz = nc.dram_tensor("bz", [F], FD, kind="ExternalInput").ap()
        need.append("bz")
    if not br_zero:
        br = nc.dram_tensor("br", [F], FD, kind="ExternalInput").ap()
        need.append("br")
    if not bh_zero:
        bh = nc.dram_tensor("bh", [F], FD, kind="ExternalInput").ap()
        need.append("bh")
    out = nc.dram_tensor("h_new", [BC, F], FD, kind="ExternalOutput").ap()

    w_r = w.rearrange("w (k p) u -> p w k u", p=P)

    def fmaj(pool, dram_ap, name):
        # [F] vector -> [128, KC] feature-major ([p, c] = v[c*128+p])
        t = pool.tile([P, KC], FD, name=name)
        nc.gpsimd.dma_start(out=t, in_=dram_ap.rearrange("(c p) -> p c", p=P))
        return t

    with tile.TileContext(nc) as tc:
        with ExitStack() as ctx:
            consts = ctx.enter_context(tc.tile_pool(name="consts", bufs=1))
            wk = ctx.enter_context(
                tc.tile_pool(name="wk", bufs=_TUNE["wkb_bufs"]))
            io_pool = ctx.enter_context(
                tc.tile_pool(name="io", bufs=_TUNE["iob_bufs"]))
            pg = ctx.enter_context(
                tc.tile_pool(name="pg", bufs=_TUNE["pgb_bufs"], space="PSUM"))
            pt = ctx.enter_context(
                tc.tile_pool(name="pt", bufs=_TUNE["ptb_bufs"], space="PSUM"))

            ident = consts.tile([P, P], BD)
            make_identity(nc, ident)
            w_sb = consts.tile([P, 6, KC, F], BD)
            pending_w = {0: [0, 1, 2, 3], 1: [4, 5]}

            gxb = ghb = mub = nmub = bzb = brb = bhb = None
            if gxr is not None:
                gxb = fmaj(consts, gxr, "gxb")
            if ghr is not None:
                ghb = fmaj(consts, ghr, "ghb")
            if mu is not None:
                mub = fmaj(consts, mu, "mub")
                nmub = consts.tile([P, KC], FD, name="nmub")
                nc.vector.tensor_scalar(nmub, mub, -1.0, None,
                                        mybir.AluOpType.mult)
            if bz is not None:
                bzb = fmaj(consts, bz, "bzb")   # host pre-scaled by 0.5
            if br is not None:
                brb = fmaj(consts, br, "brb")   # host pre-scaled by 0.5
            if bh is not None:
                bhb = fmaj(consts, bh, "bhb")

            def s_load(c):
                """DMA-transpose x/m/dt/h for batch rows [c*CW, (c+1)*CW)."""
                rows = slice(c * CW, (c + 1) * CW)
                xT = io_pool.tile([P, KC, CW], BD, tag="xT")
                mT = io_pool.tile([P, KC, CW], BD, tag="mT")
                dT = io_pool.tile([P, KC, CW], BD, tag="dT")
                hT = io_pool.tile([P, KC, CW], BD, tag="hT")
                # delta_t first (the exp is the first consumer), then m, x, h
                for f in range(KC):
                    nc.sync.dma_start(
                        out=dT[:, f, :],
                        in_=inp[rows, 2 * F + f * P:2 * F + (f + 1) * P],
                        transpose=True)
                for f in range(KC):
                    nc.sync.dma_start(out=mT[:, f, :],
                                      in_=inp[rows, F + f * P:F + (f + 1) * P],
                                      transpose=True)
                for f in range(KC):
                    nc.sync.dma_start(out=xT[:, f, :],
                                      in_=inp[rows, ts(f, P)], transpose=True)
                for f in range(KC):
                    nc.sync.dma_start(out=hT[:, f, :],
                                      in_=hp[rows, ts(f, P)], transpose=True)
                return dict(xT=xT, mT=mT, dT=dT, hT=hT, c=c)

            def s_imp(st):
                """Imputation + decay, feature-major, mostly in place:
                xT -> x_decayed, hT -> h_decayed, mT/dT scratch."""
                xT, mT, dT, hT = st["xT"], st["mT"], st["dT"], st["hT"]
                g = wk.tile([P, KC, CW], BD, tag="g")
                if gx_one:
                    nc.scalar.activation(g, dT, AF.Exp, scale=-1.0)
                else:
                    for f in range(KC):
                        nc.scalar.activation(g[:, f, :], dT[:, f, :], AF.Exp,
                                             scale=gxb[:, f:f + 1])
                if shared_gamma:
                    gH = g
                elif gh_one:
                    gH = wk.tile([P, KC, CW], BD, tag="gH")
                    nc.scalar.activation(gH, dT, AF.Exp, scale=-1.0)
                else:
                    gH = wk.tile([P, KC, CW], BD, tag="gH")
                    for f in range(KC):
                        nc.scalar.activation(gH[:, f, :], dT[:, f, :], AF.Exp,
                                             scale=ghb[:, f:f + 1])
                # q = m + g - m*g  (dT <- m*g, mT <- q)
                nc.vector.tensor_mul(dT, mT, g)
                nc.vector.tensor_add(mT, mT, g)
                nc.vector.tensor_sub(mT, mT, dT)
                # xd = x*q (+ mu*(1-q))
                nc.vector.tensor_mul(xT, xT, mT)
                if not mu_zero:
                    for f in range(KC):
                        nc.vector.tensor_scalar(
                            dT[:, f, :], mT[:, f, :],
                            nmub[:, f:f + 1], mub[:, f:f + 1],
                            mybir.AluOpType.mult, mybir.AluOpType.add)
                    nc.vector.tensor_add(xT, xT, dT)
                # hd = gH * h
                nc.vector.tensor_mul(hT, gH, hT)
                st["g"] = g
                return st

            def s_zr(st):
                """z and r gates + rh (in place on rt)."""
                xT, hT = st["xT"], st["hT"]
                zt = wk.tile([P, KC, CW], BD, tag="zt")
                for u in range(KC):
                    zp = pg.tile([P, CW], FD, tag="gp")
                    for f in range(KC):
                        nc.tensor.matmul(zp, w_sb[:, 0, f, ts(u, P)],
                                         xT[:, f, :],
                                         start=(f == 0), stop=False)
                    for f in range(KC):
                        nc.tensor.matmul(zp, w_sb[:, 1, f, ts(u, P)],
                                         hT[:, f, :],
                                         start=False, stop=(f == KC - 1))
                    if bz_zero:
                        nc.scalar.activation(zt[:, u, :], zp, AF.Tanh,
                                             scale=0.5)
                    else:
                        nc.scalar.activation(zt[:, u, :], zp, AF.Tanh,
                                             scale=0.5, bias=bzb[:, u:u + 1])
                z = wk.tile([P, KC, CW], BD, tag="z")
                getattr(nc, _TUNE["zaff_eng"]).tensor_scalar(
                    z, zt, 0.5, 0.5,
                    mybir.AluOpType.mult, mybir.AluOpType.add)
                rt = wk.tile([P, KC, CW], BD, tag="rt")
                for u in range(KC):
                    rp = pg.tile([P, CW], FD, tag="gp")
                    for f in range(KC):
                        nc.tensor.matmul(rp, w_sb[:, 2, f, ts(u, P)],
                                         xT[:, f, :],
                                         start=(f == 0), stop=False)
                    for f in range(KC):
                        nc.tensor.matmul(rp, w_sb[:, 3, f, ts(u, P)],
                                         hT[:, f, :],
                                         start=False, stop=(f == KC - 1))
                    if br_zero:
                        nc.scalar.activation(rt[:, u, :], rp, AF.Tanh,
                                             scale=0.5)
                    else:
                        nc.scalar.activation(rt[:, u, :], rp, AF.Tanh,
                                             scale=0.5, bias=brb[:, u:u + 1])
                # rh' = hd + rt*hd  (U_h host-scaled by 0.5); in place on rt
                nc.vector.tensor_mul(rt, rt, hT)
                nc.vector.tensor_add(rt, rt, hT)
                st.update(z=z, rh=rt)
                return st

            def s_hh(st):
                """hh gate, blend (in place -> h_new^T), transpose, store."""
                xT, hT, rh, z, c = st["xT"], st["hT"], st["rh"], st["z"], st["c"]
                hhT = wk.tile([P, KC, CW], BD, tag="hhT")
                for u in range(KC):
                    hpp = pg.tile([P, CW], FD, tag="gp")
                    for f in range(KC):
                        nc.tensor.matmul(hpp, w_sb[:, 4, f, ts(u, P)],
                                         xT[:, f, :],
                                         start=(f == 0), stop=False)
                    for f in range(KC):
                        nc.tensor.matmul(hpp, w_sb[:, 5, f, ts(u, P)],
                                         rh[:, f, :],
                                         start=False, stop=(f == KC - 1))
                    if bh_zero:
                        nc.scalar.activation(hhT[:, u, :], hpp, AF.Tanh)
                    else:
                        nc.scalar.activation(hhT[:, u, :], hpp, AF.Tanh,
                                             bias=bhb[:, u:u + 1])
                # h_new = hd + z*(hh - hd), in place on hhT
                nc.vector.tensor_sub(hhT, hhT, hT)
                nc.vector.tensor_mul(hhT, z, hhT)
                nc.vector.tensor_add(hhT, hT, hhT)
                # transpose back to batch-major and store
                for bs in range(CW // P):
                    hn_p = pt.tile([P, F], BD, tag="tp")
                    for u in range(KC):
                        nc.tensor.transpose(hn_p[:, ts(u, P)],
                                            hhT[:, u, ts(bs, P)], ident)
                    hn = wk.tile([P, F], FD, tag="hn")
                    _copy(nc, _TUNE["hnc_eng"], hn, hn_p)
                    nc.sync.dma_start(
                        out=out[c * CW + bs * P:c * CW + (bs + 1) * P, :],
                        in_=hn)

            if reps > 1:
                loop_ctx = tc.For_i(0, reps, 1)
                loop_ctx.__enter__()

            stages = [s_load, s_imp, s_zr, s_hh]
            offs = _TUNE["offs_b"]
            sts = {}
            for r in range(NCH + max(offs)):
                if r > 0:
                    for j in pending_w.pop(r - 1, ()):
                        nc.sync.dma_start(out=w_sb[:, j], in_=w_r[:, j])
                for s, off in zip(stages, offs):
                    j = r - off
                    if 0 <= j < NCH:
                        if s is s_load:
                            sts[j] = s(j)
                        else:
                            s(sts[j])
            sts.clear()

            if reps > 1:
                loop_ctx.__exit__(None, None, None)

    nc.compile()
    return nc, need


class _Runner:
    """Cached jitted shard_map executor (mirrors bass2jax.run_bass_via_pjrt).

    Inputs are passed as FULL concatenated arrays (shape[0] = 8x the
    per-core shard), which shard_map slices along axis 0 — a per-core
    shard list concatenated on axis 0 is just the original full tensor.
    """

    def __init__(self, nc):
        import jax
        from concourse import bass2jax, mybir

        bass2jax.install_neuronx_cc_hook()

        partition_name = (nc.partition_id_tensor.name
                          if nc.partition_id_tensor else None)
        in_names, out_names, out_avals = [], [], []
        for alloc in nc.m.functions[0].allocations:
            if not isinstance(alloc, mybir.MemoryLocationSet):
                continue
            name = alloc.memorylocations[0].name
            if alloc.kind == "ExternalInput":
                if name != partition_name:
                    in_names.append(name)
            elif alloc.kind == "ExternalOutput":
                out_names.append(name)
                out_avals.append(jax.core.ShapedArray(
                    tuple(alloc.tensor_shape), mybir.dt.np(alloc.dtype)))
        n_params = len(in_names)
        n_outs = len(out_names)
        all_names = tuple(in_names) + tuple(out_names)
        if partition_name is not None:
            all_names = all_names + (partition_name,)

        def _body(*args):
            operands = list(args)
            if partition_name is not None:
                operands.append(bass2jax.partition_id_tensor())
            outs = bass2jax._bass_exec_p.bind(
                *operands,
                out_avals=tuple(out_avals),
                in_names=all_names,
                out_names=tuple(out_names),
                lowering_input_output_aliases=(),
                sim_require_finite=True,
                sim_require_nnan=True,
                nc=nc,
            )
            return tuple(outs)

        devices = jax.devices()[:N_CORES]
        self.mesh = bass2jax.Mesh(np.asarray(devices), ("core",))
        self.spec = bass2jax.PartitionSpec("core")
        rep = bass2jax.PartitionSpec()
        # inp/hp are batch-sharded; everything else is replicated.
        self.in_specs = tuple(
            self.spec if name in ("inp", "hp") else rep for name in in_names)
        self.sharded = jax.jit(
            bass2jax.shard_map(
                _body, mesh=self.mesh,
                in_specs=self.in_specs + (self.spec,) * n_outs,
                out_specs=(self.spec,) * n_outs,
                check_rep=False),
            donate_argnums=tuple(range(n_params, n_params + n_outs)),
            keep_unused=True,
        )
        self.in_names = in_names
        self.out_names = out_names
        self.out_avals = out_avals

    def zeros(self):
        return [np.zeros((N_CORES * a.shape[0], *a.shape[1:]), a.dtype)
                for a in self.out_avals]

    def __call__(self, full_arrays):
        """full_arrays: dict name -> full (8x-shard-concat) np array."""
        concat_in = [full_arrays[name] for name in self.in_names]
        out_arrs = self.sharded(*concat_in, *self.zeros())
        return np.asarray(out_arrs[0])


def _get_runner(flags, reps=1):
    with _lock:
        key = (flags, reps)
        if key not in _cache:
            nc, need = _build_nc(flags, reps)
            _cache[key] = (_Runner(nc), need)
        return _cache[key]


def _host_arrays(inputs):
    """Host-side value prep shared by kernel() and the test harnesses.
    Returns (flags, full_arrays dict keyed by dram-tensor name) — extras
    included unconditionally; callers filter by the program's `need`."""
    inp = np.ascontiguousarray(inputs["inputs"], dtype=np.float32)
    hp = np.ascontiguousarray(inputs["h_prev"], dtype=np.float32)
    W = np.stack([np.asarray(inputs[k], dtype=np.float32)
                  for k in ("W_z", "U_z", "W_r", "U_r", "W_h", "U_h")])
    # The r-gate is computed as rt = tanh(r_pre/2) with rh' = hd + rt*hd
    # = 2*(r*hd); the missing 0.5 of sigmoid(a) = 0.5 + 0.5*tanh(a/2) is
    # folded into U_h here (rh' only ever feeds rh' @ U_h).
    W[5] *= 0.5
    gxr = np.maximum(np.asarray(inputs["gamma_x_decay"], np.float32), 0.0)
    ghr = np.maximum(np.asarray(inputs["gamma_h_decay"], np.float32), 0.0)
    mu = np.asarray(inputs["mean_imputation"], np.float32)
    bz = np.asarray(inputs["b_z"], np.float32)
    br = np.asarray(inputs["b_r"], np.float32)
    bh = np.asarray(inputs["b_h"], np.float32)

    gx_one = bool(np.all(gxr == 1.0))
    gh_one = bool(np.all(ghr == 1.0))
    shared = bool(np.array_equal(gxr, ghr))
    flags = (shared, gx_one, gh_one,
             not mu.any(), not bz.any(), not br.any(), not bh.any())

    full = {"inp": _to_bf16(inp), "hp": _to_bf16(hp), "w": _to_bf16(W)}
    if _TUNE.get("layout", "A") == "B":
        # B contract: decays are negated fp32 ACT scales; z/r biases are
        # pre-scaled by 0.5 (fused into tanh(a/2 + b/2)).
        full["gxr"] = -gxr
        full["ghr"] = -ghr
        full["mu"] = mu
        full["bz"] = 0.5 * bz
        full["br"] = 0.5 * br
        full["bh"] = bh
    else:
        full["gxr"] = _to_bf16(gxr)
        full["ghr"] = _to_bf16(ghr)
        full["mu"] = _to_bf16(mu)
        full["bz"] = bz
        full["br"] = br
        full["bh"] = bh
    return flags, full


def _prep(inputs):
    """Returns (runner, full_arrays dict keyed by dram-tensor name)."""
    flags, full = _host_arrays(inputs)
    run, need = _get_runner(flags)
    return run, {k: v for k, v in full.items()
                 if k in ("inp", "hp", "w") or k in need}


def kernel(**inputs) -> np.ndarray:
    run, full = _prep(inputs)
    return run(full)


def _timed_calls(run, full, iters):
    import time

    import jax
    from jax.sharding import NamedSharding

    out_sh = NamedSharding(run.mesh, run.spec)
    dev_in = [
        jax.device_put(full[name], NamedSharding(run.mesh, spec))
        for name, spec in zip(run.in_names, run.in_specs)
    ]
    zero_sets = [[jax.device_put(z, out_sh) for z in run.zeros()]
                 for _ in range(iters)]
    jax.block_until_ready(dev_in)
    jax.block_until_ready(zero_sets)
    jax.block_until_ready(run.sharded(*dev_in, *run.zeros()))  # warm-up
    times = []
    for i in range(iters):
        t0 = time.perf_counter_ns()
        out = run.sharded(*dev_in, *zero_sets[i])
        jax.block_until_ready(out)
        times.append(time.perf_counter_ns() - t0)
    times.sort()
    return times


def bench_device(inputs, iters: int = 14, reps_lo: int = 8, reps_hi: int = 64):
    """Estimate on-device per-pass execution time via loop-delta timing:
    two looped variants of the kernel (hardware For-loop in one NEFF, same
    code shape) amortize the large axon dispatch overhead; per-pass time =
    (median(T_hi) - median(T_lo)) / (reps_hi - reps_lo). Medians because
    the dispatch noise has heavy two-sided outliers. The loop back-edge is
    a full barrier, so this measures a complete pass including pipeline
    fill/drain — a slight over-estimate of the streamed single-shot time."""
    run1, full = _prep(inputs)
    flags_key = next(k for k in _cache if _cache[k][0] is run1)[0]
    run_lo, _ = _get_runner(flags_key, reps=reps_lo)
    run_hi, _ = _get_runner(flags_key, reps=reps_hi)

    t_lo = _timed_calls(run_lo, full, iters)
    t_hi = _timed_calls(run_hi, full, iters)
    med_lo = t_lo[len(t_lo) // 2]
    med_hi = t_hi[len(t_hi) // 2]
    per_pass = (med_hi - med_lo) / (reps_hi - reps_lo)
    return {
        "per_pass_ns": int(per_pass),
        "t_lo_med_ns": med_lo,
        "t_hi_med_ns": med_hi,
        "t_lo_all": t_lo,
        "t_hi_all": t_hi,
    }
